# revision 1
# baseline (speedup 1.0000x reference)
"""Multi-head self-attention (B=4, S=2048, E=1024, H=16) on 8 TRN2 NeuronCores.

Sharding: 8 cores = 4 batches x 2 sequence halves. Core c handles batch b=c//2,
query rows [h*1024, (h+1)*1024) with h=c%2. Each core computes Q/K/V for its own
1024-row shard, the K/V shards are exchanged via one 8-rank AllGather (bf16,
K^T and V concatenated in one bounce buffer), and each core then runs full
attention for its 16 heads x 1024 queries over the 2048 keys of its batch,
followed by the full output projection for its rows. Host only shards inputs,
transposes/casts, and concatenates the per-core outputs.

Math notes (exactness-preserving rewrites):
- K bias dropped: adds a per-query constant to every score -> softmax invariant.
- V bias folded into the output-projection bias: bo_eff = WO @ bV + bO.
- 1/sqrt(D) and the additive key mask (-1e6 on masked keys) are fused into the
  exp activation: p = Exp(score/8 + negmask[key]).
- No max-subtraction in softmax: scores are O(1) here, exp cannot overflow.
- Softmax normalizer l rides as a ones-column in the V-hat stationary tiles;
  normalization is applied to the attention output (commutes with per-query
  scaling), via a K=1 ones-matmul that broadcasts 1/l across partitions.
"""

import sys
import os

if "/opt/trn_rl_repo" not in sys.path:
    sys.path.insert(0, "/opt/trn_rl_repo")

import numpy as np
import ml_dtypes

import concourse.bass as bass
import concourse.mybir as mybir
from concourse import bacc
from concourse.tile import TileContext
from concourse.bass_utils import run_bass_kernel_spmd

BF16 = mybir.dt.bfloat16
F32 = mybir.dt.float32

B, S, E, H = 4, 2048, 1024, 16
D = E // H          # 64
N_CORES = 8
ROWS = S // 2       # 1024 query rows per core
KEYS = S            # 2048 keys per core (full batch)
KT = E // 128       # 8 contraction tiles
JT = E // 128       # 8 output-feature tiles
ET = E // 128       # 8 e-tiles (head pairs)
NJC = KEYS // 128   # 16 key chunks
NIC = ROWS // 512   # 2 query chunks of 512
NVT = KEYS // 128   # 16 v key-tiles
NIT = ROWS // 128   # 8 query row-tiles
SCALE = 1.0 / 8.0   # 1/sqrt(D)

_prog_cache = {}


def _build_program(sim=False, loop_n=0):
    """sim=True builds a single-core variant for TimelineSim: the AllGathers are
    dropped and ag_k/ag_v become plain internal DRAM tensors (timing-only).
    loop_n>0 (requires sim=True) wraps the WO phase in a hardware For_i loop
    for wall-clock amplification benchmarks."""
    assert not loop_n or sim
    nc = bacc.Bacc("TRN2", target_bir_lowering=False, debug=False, num_devices=N_CORES)

    xT = nc.dram_tensor("xT", [E, ROWS], BF16, kind="ExternalInput").ap()
    wq = nc.dram_tensor("wq", [E, E], BF16, kind="ExternalInput").ap()
    wk = nc.dram_tensor("wk", [E, E], BF16, kind="ExternalInput").ap()
    wv = nc.dram_tensor("wv", [E, E], BF16, kind="ExternalInput").ap()
    wo = nc.dram_tensor("wo", [E, E], BF16, kind="ExternalInput").ap()
    bq = nc.dram_tensor("bq", [128, JT], F32, kind="ExternalInput").ap()
    negmask = nc.dram_tensor("negmask", [128, NJC], F32, kind="ExternalInput").ap()
    outmask = nc.dram_tensor("outmask", [128, NIT], F32, kind="ExternalInput").ap()
    bo_eff = nc.dram_tensor("bo_eff", [1, E], BF16, kind="ExternalInput").ap()
    pair_base = nc.dram_tensor("pair_base", [1, 1], mybir.dt.uint32, kind="ExternalInput").ap()
    out = nc.dram_tensor("out", [ROWS, E], F32, kind="ExternalOutput").ap()

    with TileContext(nc) as tc:
        with (
            tc.tile_pool(name="persist", bufs=1) as persist,
            tc.tile_pool(name="dram", bufs=1, space="DRAM") as dram,
        ):
            # ---- persistent small tensors ----
            bq_t = persist.tile([128, JT], F32)
            nc.sync.dma_start(out=bq_t, in_=bq[:, :])
            nm_t = persist.tile([128, NJC], F32)
            nc.sync.dma_start(out=nm_t, in_=negmask[:, :])
            om_t = persist.tile([128, NIT], F32)
            nc.sync.dma_start(out=om_t, in_=outmask[:, :])
            ones_t = persist.tile([1, 128], BF16)
            nc.vector.memset(ones_t, 1.0)
            # ---- persistent big tensors ----
            ao_sb = [persist.tile([128, ROWS], BF16, name=f"ao{t}") for t in range(ET)]
            qT_sb = [persist.tile([128, ROWS], BF16, name=f"qT{j}") for j in range(JT)]
            # per-(pair,head) softmax denominators: head hh at partition hh*64
            # (bases 0 and 64 are both legal; footprint = one free-dim range)
            la_sb = [persist.tile([D + 1, ROWS], BF16, name=f"la{t}") for t in range(ET)]

            # ---- bounce buffers for the K/V exchange ----
            addr_space = "Local" if sim else "Shared"
            bounce_k = dram.tile([ROWS, ROWS], BF16)   # own K^T shard (feature-major)
            bounce_v = dram.tile([ROWS, ROWS], BF16)   # own V shard (row-major)
            ag_k = dram.tile([N_CORES * ROWS, ROWS], BF16, addr_space=addr_space)
            ag_v = dram.tile([N_CORES * ROWS, ROWS], BF16, addr_space=addr_space)

            # Key order on this core: [own 1024 | partner 1024] (contraction over
            # keys is permutation-invariant; host reorders negmask to match).
            # Attention runs in two sweeps: sweep A = all pairs x own keys
            # (starts as soon as K(0)/Q(0) exist; K/V/Q production interleaves
            # into the early pairs), sweep B = all pairs x partner keys (the
            # AllGathers complete far behind sweep A's ~130us of work).
            # Sweep A leaves unnormalized partials in ao_sb and raw l in rl_sb;
            # sweep B accumulates, then 1/l is broadcast via a K=2 matmul.
            with (
                tc.tile_pool(name="p_xq", bufs=1) as p_xq,    # x + WQ
                tc.tile_pool(name="p_kst", bufs=8) as p_kst,  # K^T slots (own/partner share)
                tc.tile_pool(name="p_vh", bufs=1) as p_vh,    # all 16 V-hat tiles
                tc.tile_pool(name="p_w", bufs=1) as p_w,      # WK + WV
                tc.tile_pool(name="p2s", bufs=3) as p2s,      # transients
                tc.tile_pool(name="psA", bufs=1, space="PSUM") as psA,
            ):
                xt = [p_xq.tile([128, ROWS], BF16, name=f"xt{k}") for k in range(KT)]
                wo_sb = [p_xq.tile([128, E], BF16, name=f"wo{k}") for k in range(KT)]
                wq_sb = [p_xq.tile([128, E], BF16, name=f"wq{k}") for k in range(KT)]
                kstage = [p_kst.tile([128, ROWS], BF16, name=f"kst{j}", tag="kst") for j in range(JT)]
                vhat = [p_vh.tile([128, H, D + 1], BF16, name=f"vh{v}") for v in range(NVT)]
                wk_sb = [p_w.tile([128, E], BF16, name=f"wk{k}") for k in range(KT)]
                wv_sb = [p_w.tile([128, E], BF16, name=f"wv{k}") for k in range(KT)]
                kpart = [p_kst.tile([128, ROWS], BF16, name=f"kp{j}", tag="kst") for j in range(JT)]

                # load order follows first use: x+WK, WQ, WV
                for k in range(KT):
                    nc.sync.dma_start(out=xt[k], in_=xT[k * 128:(k + 1) * 128, :])
                    nc.sync.dma_start(out=wk_sb[k], in_=wk[k * 128:(k + 1) * 128, :])
                for k in range(KT):
                    nc.sync.dma_start(out=wq_sb[k], in_=wq[k * 128:(k + 1) * 128, :])
                for k in range(KT):
                    nc.sync.dma_start(out=wv_sb[k], in_=wv[k * 128:(k + 1) * 128, :])

                # "s" slots ([128,1024] = 2 PSUM banks x 2) host every transient
                # accumulation: K/V/Q projections, score tiles, normalization.
                # "av" slots (1 bank x 4) host the 4 attn@v chains of a pair.
                def s_tile(shape=None):
                    return psA.tile(shape or [128, ROWS], F32, name="ps_s", tag="s", bufs=2)

                def emit_k(j):
                    for ic in range(NIC):
                        sl = slice(ic * 512, (ic + 1) * 512)
                        ps_k = s_tile([128, 512])
                        for k in range(KT):
                            nc.tensor.matmul(
                                ps_k, wk_sb[k][:, j * 128:(j + 1) * 128], xt[k][:, sl],
                                start=(k == 0), stop=(k == KT - 1),
                            )
                        nc.vector.tensor_copy(kstage[j][:, sl], ps_k)
                    nc.sync.dma_start(out=bounce_k[j * 128:(j + 1) * 128, :], in_=kstage[j])

                def emit_q(j):
                    for ic in range(NIC):
                        sl = slice(ic * 512, (ic + 1) * 512)
                        ps_q = s_tile([128, 512])
                        for k in range(KT):
                            nc.tensor.matmul(
                                ps_q, wq_sb[k][:, j * 128:(j + 1) * 128], xt[k][:, sl],
                                start=(k == 0), stop=(k == KT - 1),
                            )
                        nc.vector.tensor_scalar_add(
                            qT_sb[j][:, sl], ps_q, bq_t[:, j:j + 1]
                        )

                def emit_v(v):
                    # V row-tile v (own keys v*128..): psum -> vhat directly
                    for jc in range(NIC):
                        sl = slice(jc * 512, (jc + 1) * 512)
                        ps_v = s_tile([128, 512])
                        for k in range(KT):
                            nc.tensor.matmul(
                                ps_v, xt[k][:, v * 128:(v + 1) * 128], wv_sb[k][:, sl],
                                start=(k == 0), stop=(k == KT - 1),
                            )
                        nc.vector.tensor_copy(
                            vhat[v][:, 8 * jc:8 * (jc + 1), 0:D],
                            ps_v.rearrange("p (h d) -> p h d", h=8),
                        )
                    nc.vector.memset(vhat[v][:, :, D:D + 1], 1.0)
                    nc.sync.dma_start(
                        out=bounce_v[v * 128:(v + 1) * 128, :],
                        in_=vhat[v][:, :, 0:D],
                    )

                def emit_partner_v(v):
                    vtmp = p2s.tile([128, E], BF16, name="vtmp", tag="vtmp", bufs=2)
                    nc.sync.dma_start(
                        out=vtmp, in_=ag_v[bass.ds(base + v * 128, 128), :]
                    )
                    nc.vector.tensor_copy(
                        vhat[NVT // 2 + v][:, :, 0:D],
                        vtmp.rearrange("p (h d) -> p h d", h=H),
                    )
                    nc.vector.memset(vhat[NVT // 2 + v][:, :, D:D + 1], 1.0)

                def emit_norm(j):
                    # normalization of pair j: 1/l broadcast across partitions
                    # via a K=1 ones-matmul, then scale ao_sb in place.
                    for hh in range(2):
                        for ic in range(NIC):
                            sl = slice(ic * 512, (ic + 1) * 512)
                            r32 = p2s.tile([1, 512], F32, name="r32", tag="r32", bufs=2)
                            nc.vector.reciprocal(r32, la_sb[j][hh * D:hh * D + 1, sl])
                            rbf = p2s.tile([1, 512], BF16, name="rbf", tag="rbf", bufs=2)
                            nc.vector.tensor_copy(rbf, r32)
                            ps_b = s_tile([D, 512])
                            nc.tensor.matmul(
                                ps_b, ones_t[:, 0:D], rbf, start=True, stop=True
                            )
                            nc.vector.tensor_mul(
                                ao_sb[j][hh * D:(hh + 1) * D, sl],
                                ao_sb[j][hh * D:(hh + 1) * D, sl],
                                ps_b,
                            )

                def emit_scores_exp(t, jc, kt_src, kcol):
                    phs = []
                    for hh in range(2):
                        prows = slice(hh * D, (hh + 1) * D)
                        ps_s = s_tile()
                        for ic in range(NIC):
                            nc.tensor.matmul(
                                ps_s[:, ic * 512:(ic + 1) * 512],
                                kt_src[prows, kcol:kcol + 128],
                                qT_sb[t][prows, ic * 512:(ic + 1) * 512],
                                start=True, stop=True,
                                tile_position=(hh * D, 0),
                            )
                        ph = p2s.tile([128, ROWS], BF16, name="ph", tag="ph", bufs=6)
                        nc.scalar.activation(
                            ph, ps_s, mybir.ActivationFunctionType.Exp,
                            bias=nm_t[:, jc:jc + 1], scale=SCALE,
                        )
                        phs.append(ph)
                    return phs

                def emit_av(t, jc, phs, ps_av, first, last):
                    for hh in range(2):
                        h = 2 * t + hh
                        for ic in range(NIC):
                            nc.tensor.matmul(
                                ps_av[hh, ic],
                                vhat[jc][:, h, :],
                                phs[hh][:, ic * 512:(ic + 1) * 512],
                                start=first, stop=last,
                            )

                emit_k(0)
                emit_q(0)
                base_reg = nc.sync.alloc_register("base_reg")
                nc.sync.reg_load(base_reg, pair_base[0:1, 0:1])
                base = nc.sync.snap(
                    base_reg, donate=True, min_val=0, max_val=(N_CORES - 1) * ROWS
                )

                # K(j>=1), Q(j>=1), V(all) interleave into sweep A's early pairs:
                # (pair, jc) -> list of emissions. V(jc) rides pair 0 exactly one
                # LAG ahead of its av consumer; K two per pair in pairs 0-2 (so
                # AG_K fires early); Q(t+2) two pairs ahead of use.
                pre = {}
                pre[(0, 1)] = [("k", 1)]
                pre[(0, 2)] = [("q", 1)]
                for j in range(2, JT):
                    pre.setdefault(((j - 2) // 2, 3 + 4 * ((j - 2) % 2)), []).append(("k", j))
                for j in range(2, JT):
                    pre.setdefault((j - 2, 6), []).append(("q", j))
                for v in range(ROWS // 128):
                    pre.setdefault((0, min(v + 1, NJC // 2 - 1)), []).append(("v", v))

                LAG = 2
                # =============== sweep A: own keys ===============
                for t in range(ET):
                    pend = []
                    ps_av = {}
                    for hh in range(2):
                        for ic in range(NIC):
                            ps_av[hh, ic] = psA.tile(
                                [D + 1, 512], F32, name="ps_av", tag="av", bufs=4
                            )
                    for jc in range(NJC // 2):
                        for kind, idx in pre.get((t, jc), ()):
                            if kind == "k":
                                emit_k(idx)
                                if idx == JT - 1:
                                    if not sim:
                                        nc.gpsimd.collective_compute(
                                            "AllGather", mybir.AluOpType.bypass,
                                            ins=[bounce_k[:, :]], outs=[ag_k[:, :]],
                                            replica_groups=[list(range(N_CORES))],
                                        )
                                    for j in range(JT):
                                        nc.sync.dma_start(
                                            out=kpart[j],
                                            in_=ag_k[bass.ds(base + j * 128, 128), :],
                                        )
                            elif kind == "q":
                                emit_q(idx)
                            else:
                                emit_v(idx)
                                if idx == ROWS // 128 - 1:
                                    if not sim:
                                        nc.gpsimd.collective_compute(
                                            "AllGather", mybir.AluOpType.bypass,
                                            ins=[bounce_v[:, :]], outs=[ag_v[:, :]],
                                            replica_groups=[list(range(N_CORES))],
                                        )
                                    for v in range(NVT // 2):
                                        emit_partner_v(v)
                        pend.append((jc, emit_scores_exp(t, jc, kstage[t], jc * 128)))
                        if len(pend) > LAG:
                            j0, phs0 = pend.pop(0)
                            emit_av(t, j0, phs0, ps_av, j0 == 0, j0 == NJC // 2 - 1)
                    for j0, phs0 in pend:
                        emit_av(t, j0, phs0, ps_av, j0 == 0, j0 == NJC // 2 - 1)
                    # stash unnormalized partials + raw l
                    for hh in range(2):
                        for ic in range(NIC):
                            sl = slice(ic * 512, (ic + 1) * 512)
                            av = ps_av[hh, ic]
                            nc.vector.tensor_copy(
                                la_sb[t][hh * D:hh * D + 1, sl], av[D:D + 1, :]
                            )
                            nc.vector.tensor_copy(
                                ao_sb[t][hh * D:(hh + 1) * D, sl], av[0:D, :]
                            )

                # =============== sweep B: partner keys ===============
                for t in range(ET):
                    pend = []
                    ps_av = {}
                    for hh in range(2):
                        for ic in range(NIC):
                            ps_av[hh, ic] = psA.tile(
                                [D + 1, 512], F32, name="ps_av", tag="av", bufs=4
                            )
                    for jc in range(NJC // 2, NJC):
                        if jc == NJC // 2 + 3 and t > 0:
                            emit_norm(t - 1)  # previous pair: off the boundary
                        pend.append((jc, emit_scores_exp(t, jc, kpart[t], (jc - NJC // 2) * 128)))
                        if len(pend) > LAG:
                            j0, phs0 = pend.pop(0)
                            emit_av(t, j0, phs0, ps_av, j0 == NJC // 2, j0 == NJC - 1)
                    for j0, phs0 in pend:
                        emit_av(t, j0, phs0, ps_av, j0 == NJC // 2, j0 == NJC - 1)
                    # accumulate into partials; l += lB (in place, frees av fast)
                    for hh in range(2):
                        for ic in range(NIC):
                            sl = slice(ic * 512, (ic + 1) * 512)
                            av = ps_av[hh, ic]
                            nc.vector.tensor_add(
                                la_sb[t][hh * D:hh * D + 1, sl],
                                la_sb[t][hh * D:hh * D + 1, sl],
                                av[D:D + 1, :],
                            )
                            nc.vector.tensor_add(
                                ao_sb[t][hh * D:(hh + 1) * D, sl],
                                ao_sb[t][hh * D:(hh + 1) * D, sl],
                                av[0:D, :],
                            )
                emit_norm(ET - 1)

                # WO weights (loaded during attention), then the output
                # projection in the same pool/tag space -- no pool barrier.
                for k in range(KT):
                    nc.sync.dma_start(out=wo_sb[k], in_=wo[k * 128:(k + 1) * 128, :])

                # ======= output projection in the same pools (no barrier) =======
                bo_t = p2s.tile([1, E], BF16, name="bo_t", tag="bo", bufs=1)
                nc.sync.dma_start(out=bo_t, in_=bo_eff[:, :])

                def emit_wo():
                    emit_wo_body(nc, tc, psA, p2s, ao_sb, wo_sb, bo_t, ones_t, om_t, out)

                if loop_n:
                    with tc.For_i(0, loop_n, 1):
                        emit_wo()
                else:
                    emit_wo()
    nc.compile()
    return nc


def emit_wo_body(nc, tc, ps3, p3, ao_sb, wo_sb, bo_t, ones_t, om_t, out):
                for it in range(NIT):
                    for fc in range(NIC):
                        sl = slice(fc * 512, (fc + 1) * 512)
                        ps_o = ps3.tile([128, 512], F32, name="ps_o", tag="av", bufs=4)
                        for k in range(KT):
                            nc.tensor.matmul(
                                ps_o,
                                ao_sb[k][:, it * 128:(it + 1) * 128],
                                wo_sb[k][:, sl],
                                start=(k == 0), stop=False,
                            )
                        nc.tensor.matmul(
                            ps_o, ones_t[:, 0:128], bo_t[:, sl],
                            start=False, stop=True,
                        )
                        o_sb = p3.tile([128, 512], F32, name="o_sb", tag="o_sb", bufs=3)
                        nc.scalar.activation(
                            o_sb, ps_o, mybir.ActivationFunctionType.Abs,
                            scale=om_t[:, it:it + 1],
                        )
                        nc.sync.dma_start(
                            out=out[it * 128:(it + 1) * 128, sl], in_=o_sb
                        )


def _make_executor():
    """Build the Bass program once and wrap it in a cached jitted shard_map
    (adapted from concourse.bass2jax.run_bass_via_pjrt, hoisting the jit out
    of the per-call path so repeat calls don't retrace/recompile)."""
    import jax
    from jax.experimental.shard_map import shard_map
    from jax.sharding import Mesh, PartitionSpec, NamedSharding
    from concourse.bass2jax import (
        _bass_exec_p,
        install_neuronx_cc_hook,
        partition_id_tensor,
    )

    nc = _build_program()
    install_neuronx_cc_hook()
    assert nc.dbg_addr is None
    partition_name = nc.partition_id_tensor.name if nc.partition_id_tensor else None

    in_names, out_names, out_avals, zero_outs = [], [], [], []
    for alloc in nc.m.functions[0].allocations:
        if not isinstance(alloc, mybir.MemoryLocationSet):
            continue
        name = alloc.memorylocations[0].name
        if alloc.kind == "ExternalInput":
            if name != partition_name:
                in_names.append(name)
        elif alloc.kind == "ExternalOutput":
            shape = tuple(alloc.tensor_shape)
            dtype = mybir.dt.np(alloc.dtype)
            out_names.append(name)
            out_avals.append(jax.core.ShapedArray(shape, dtype))
            zero_outs.append(np.zeros(shape, dtype))
    n_params = len(in_names)
    n_outs = len(out_avals)
    all_names = in_names + out_names
    if partition_name is not None:
        all_names = all_names + [partition_name]
    donate = tuple(range(n_params, n_params + n_outs))

    def _body(*args):
        operands = list(args)
        if partition_name is not None:
            operands.append(partition_id_tensor())
        outs = _bass_exec_p.bind(
            *operands,
            out_avals=tuple(out_avals),
            in_names=tuple(all_names),
            out_names=tuple(out_names),
            lowering_input_output_aliases=(),
            sim_require_finite=True,
            sim_require_nnan=True,
            nc=nc,
        )
        return tuple(outs)

    devices = jax.devices()[:N_CORES]
    mesh = Mesh(np.asarray(devices), ("core",))
    in_specs = (PartitionSpec("core"),) * (n_params + n_outs)
    out_specs = (PartitionSpec("core"),) * n_outs
    sharded = jax.jit(
        shard_map(_body, mesh=mesh, in_specs=in_specs, out_specs=out_specs,
                  check_rep=False),
        donate_argnums=donate,
        keep_unused=True,
    )
    sharding = NamedSharding(mesh, PartitionSpec("core"))
    return {
        "jit": sharded, "in_names": in_names, "out_names": out_names,
        "out_avals": out_avals, "zero_outs": zero_outs, "sharding": sharding,
        "jax": jax,
    }


def get_executor():
    if "ex" not in _prog_cache:
        _prog_cache["ex"] = _make_executor()
    return _prog_cache["ex"]


def run_spmd(in_maps):
    """Execute on 8 cores; returns list of per-core output dicts."""
    ex = get_executor()
    jax = ex["jax"]
    concat_in = [
        np.concatenate([np.asarray(m[name]) for m in in_maps], axis=0)
        for name in ex["in_names"]
    ]
    concat_zeros = [
        np.zeros((N_CORES * z.shape[0], *z.shape[1:]), z.dtype)
        for z in ex["zero_outs"]
    ]
    out_arrs = ex["jit"](*concat_in, *concat_zeros)
    return [
        {
            name: np.asarray(out_arrs[i]).reshape(N_CORES, *ex["out_avals"][i].shape)[c]
            for i, name in enumerate(ex["out_names"])
        }
        for c in range(N_CORES)
    ]


def build_in_maps(x, mask, WQ_w, WQ_b, WK_w, WK_b, WV_w, WV_b, WO_w, WO_b):
    x = np.asarray(x, dtype=np.float32)
    mask = np.asarray(mask).astype(bool)
    WQ_w = np.asarray(WQ_w, dtype=np.float32)
    WQ_b = np.asarray(WQ_b, dtype=np.float32)
    WK_w = np.asarray(WK_w, dtype=np.float32)
    WV_w = np.asarray(WV_w, dtype=np.float32)
    WV_b = np.asarray(WV_b, dtype=np.float32)
    WO_w = np.asarray(WO_w, dtype=np.float32)
    WO_b = np.asarray(WO_b, dtype=np.float32)

    wq_t = np.ascontiguousarray(WQ_w.T).astype(ml_dtypes.bfloat16)
    wk_t = np.ascontiguousarray(WK_w.T).astype(ml_dtypes.bfloat16)
    wv_t = np.ascontiguousarray(WV_w.T).astype(ml_dtypes.bfloat16)
    wo_t = np.ascontiguousarray(WO_w.T).astype(ml_dtypes.bfloat16)
    bq_t = np.ascontiguousarray(WQ_b.reshape(JT, 128).T)  # [128, JT] f32
    bo_eff = (WO_w @ WV_b + WO_b).astype(ml_dtypes.bfloat16).reshape(1, E)

    in_maps = []
    for c in range(N_CORES):
        b, h = divmod(c, 2)
        x_sh = x[b, h * ROWS:(h + 1) * ROWS, :]                      # (1024, 1024)
        xT_sh = np.ascontiguousarray(x_sh.T).astype(ml_dtypes.bfloat16)
        # key order on this core: [own half | partner half]
        mask_perm = np.concatenate(
            [mask[b, h * ROWS:(h + 1) * ROWS], mask[b, (1 - h) * ROWS:(2 - h) * ROWS]]
        )
        negmask = np.where(mask_perm, 0.0, -1e6).astype(np.float32)
        nm_t = np.ascontiguousarray(negmask.reshape(NJC, 128).T)     # [128, 16]
        om = mask[b, h * ROWS:(h + 1) * ROWS].astype(np.float32)
        om_t = np.ascontiguousarray(om.reshape(NIT, 128).T)          # [128, 8]
        in_maps.append({
            "xT": xT_sh, "wq": wq_t, "wk": wk_t, "wv": wv_t, "wo": wo_t,
            "bq": bq_t, "negmask": nm_t, "outmask": om_t, "bo_eff": bo_eff,
            "pair_base": np.array([[(c ^ 1) * ROWS]], dtype=np.uint32),
        })
    return in_maps


def kernel(x, mask, WQ_w, WQ_b, WK_w, WK_b, WV_w, WV_b, WO_w, WO_b):
    mask = np.asarray(mask).astype(bool)
    in_maps = build_in_maps(x, mask, WQ_w, WQ_b, WK_w, WK_b, WV_w, WV_b, WO_w, WO_b)
    results = run_spmd(in_maps)
    out = np.empty((B, S, E), dtype=np.float32)
    for c in range(N_CORES):
        b, h = divmod(c, 2)
        out[b, h * ROWS:(h + 1) * ROWS, :] = results[c]["out"]
    return out



# revision 18
# speedup vs baseline: 344.9173x; 344.9173x over previous
"""Multi-head self-attention (B=4, S=2048, E=1024, H=16) on 8 TRN2 NeuronCores.

v3 strategy: mask compaction + head parallelism, no collectives.

Observation: the reference zeroes output rows where mask==0 (o * mask before
abs), and masked keys get -1e6 scores (zero softmax weight). With a ~50%
random mask, half the rows are dead. The host compacts each batch to its
valid rows and scatters results back.

Sharding: core c handles batch b=c//2 and head-group hg=c%2 (8 of 16 heads).
Each core computes K/V for all Pk compacted rows and Q for the first Pq
(=1024*nb) compacted rows of its batch, for its 8 heads (512-feature weight
slices), runs full attention, and emits the partial output projection
o_part = attn_out @ WO[:, hg]^T in bf16. The host sums the two partials per
batch, adds bo_eff = WO@bV + bO, applies abs, and scatters into the zeroed
full output. The <=64 query rows beyond Pq ("overflow") are finished on the
host in numpy, using K^T and V fetched from the device (tiny extra DMA).

Math notes (exactness-preserving rewrites):
- K bias dropped (softmax invariant); V bias folded into host-side bo_eff.
- 1/sqrt(D) and the pad-key mask (-1e6) fused into the Exp activation.
- No max-subtraction: scores are O(1), exp cannot overflow.
- Softmax denominator l rides as a ones-column in V-hat. Per pair, the raw
  attention output and l are evacuated from PSUM with two cheap copies
  (freeing the accumulator banks immediately); the reciprocal + ones-matmul
  broadcast + multiply then run off the critical path.

PSUM (8 banks): scores/projections/WO share tag 'sc' [128,1024] f32 x2 bufs
(4 banks); av accumulators [65,1024] f32 x2 (4 banks). Query blocks are
exactly 1024 wide, so each Exp activation is one (1024+352)-cycle ACTIVATE.
"""

import sys

if "/opt/trn_rl_repo" not in sys.path:
    sys.path.insert(0, "/opt/trn_rl_repo")

import numpy as np
import ml_dtypes

import concourse.bass as bass
import concourse.mybir as mybir
from concourse import bacc
from concourse.tile import TileContext

BF16 = mybir.dt.bfloat16
F32 = mybir.dt.float32

B, S, E, H = 4, 2048, 1024, 16
D = E // H          # 64
N_CORES = 8
HG = H // 2         # 8 heads per core
EG = HG * D         # 512 head-group features per core
KT = E // 128       # 8 contraction tiles over E
NJ = EG // 128      # 4 feature tiles (j) per head group
PAIRS = NJ          # 4 head pairs per core
SCALE = 1.0 / 8.0   # 1/sqrt(D)
QBLK = 1024         # query block width (PSUM-sized)
OVF = 64            # max query rows finished on the host

_prog_cache = {}


def _chunks(n, step=512):
    out, off = [], 0
    while off < n:
        w = min(step, n - off)
        out.append((off, w))
        off += w
    return out


def _build_program(Pk, Pq, want_kv):
    NKC = Pk // 128      # key chunks
    NB = Pq // QBLK      # query blocks
    nc = bacc.Bacc("TRN2", target_bir_lowering=False, debug=False, num_devices=N_CORES)

    xT = nc.dram_tensor("xT", [E, Pk], BF16, kind="ExternalInput").ap()
    wq = nc.dram_tensor("wq", [E, EG], BF16, kind="ExternalInput").ap()
    wk = nc.dram_tensor("wk", [E, EG], BF16, kind="ExternalInput").ap()
    wv = nc.dram_tensor("wv", [E, EG], BF16, kind="ExternalInput").ap()
    wo = nc.dram_tensor("wo", [EG, E], BF16, kind="ExternalInput").ap()
    bq = nc.dram_tensor("bq", [128, NJ], F32, kind="ExternalInput").ap()
    negmask = nc.dram_tensor("negmask", [128, NKC], F32, kind="ExternalInput").ap()
    out = nc.dram_tensor("out", [Pq, E], BF16, kind="ExternalOutput").ap()
    if want_kv:
        out_k = nc.dram_tensor("out_k", [EG, Pk], BF16, kind="ExternalOutput").ap()
        out_v = nc.dram_tensor("out_v", [Pk, EG], BF16, kind="ExternalOutput").ap()

    with TileContext(nc) as tc:
        with tc.tile_pool(name="persist", bufs=1) as persist:
            bq_t = persist.tile([128, NJ], F32)
            nc.sync.dma_start(out=bq_t, in_=bq[:, :])
            nm_t = persist.tile([128, NKC], F32)
            nc.sync.dma_start(out=nm_t, in_=negmask[:, :])
            ones_t = persist.tile([1, 128], BF16)
            nc.vector.memset(ones_t, 1.0)
            ones32 = persist.tile([1, 128], F32)
            nc.vector.memset(ones32, 1.0)

            xt = [persist.tile([128, Pk], BF16, name=f"xt{k}") for k in range(KT)]
            wk_sb = [persist.tile([128, EG], BF16, name=f"wk{k}") for k in range(KT)]
            wq_sb = [persist.tile([128, EG], BF16, name=f"wq{k}") for k in range(KT)]
            wv_sb = [persist.tile([128, EG], BF16, name=f"wv{k}") for k in range(KT)]
            wo_sb = [persist.tile([128, E], BF16, name=f"wo{k}") for k in range(NJ)]
            kstage = [persist.tile([128, Pk], BF16, name=f"kst{j}") for j in range(NJ)]
            qT = [persist.tile([128, Pq], BF16, name=f"qT{j}") for j in range(NJ)]
            vhat = [persist.tile([128, HG, D + 1], BF16, name=f"vh{v}") for v in range(NKC)]
            ao = [persist.tile([128, Pq], BF16, name=f"ao{t}") for t in range(PAIRS)]
            wops = [persist.tile([128, E], BF16, name=f"wop{q}")
                    for q in range(QBLK // 128)]

            for k in range(KT):
                nc.sync.dma_start(out=xt[k][:, 0:512],
                                  in_=xT[k * 128:(k + 1) * 128, 0:512])
                nc.sync.dma_start(out=wk_sb[k][:, 0:128],
                                  in_=wk[k * 128:(k + 1) * 128, 0:128])
                nc.sync.dma_start(out=wq_sb[k][:, 0:128],
                                  in_=wq[k * 128:(k + 1) * 128, 0:128])
            for k in range(KT):
                nc.sync.dma_start(out=xt[k][:, 512:Pk],
                                  in_=xT[k * 128:(k + 1) * 128, 512:Pk])
            for k in range(KT):
                nc.sync.dma_start(out=wv_sb[k], in_=wv[k * 128:(k + 1) * 128, :])
            for k in range(KT):
                nc.sync.dma_start(out=wk_sb[k][:, 128:EG],
                                  in_=wk[k * 128:(k + 1) * 128, 128:EG])
                nc.sync.dma_start(out=wq_sb[k][:, 128:EG],
                                  in_=wq[k * 128:(k + 1) * 128, 128:EG])

            with (
                tc.tile_pool(name="p2s", bufs=3) as p2s,
                tc.tile_pool(name="psA", bufs=1, space="PSUM") as psA,
            ):
                def sc_tile():
                    return psA.tile([128, 1024], F32, name="ps_sc", tag="sc", bufs=2)

                def emit_k(j):
                    for off, w in _chunks(Pk, 1024):
                        ps = sc_tile()
                        for coff, cw in _chunks(w):
                            for k in range(KT):
                                nc.tensor.matmul(
                                    ps[:, coff:coff + cw],
                                    wk_sb[k][:, j * 128:(j + 1) * 128],
                                    xt[k][:, off + coff:off + coff + cw],
                                    start=(k == 0), stop=(k == KT - 1),
                                )
                        nc.vector.tensor_copy(kstage[j][:, off:off + w], ps[:, 0:w])
                    if want_kv:
                        nc.sync.dma_start(out=out_k[j * 128:(j + 1) * 128, :],
                                          in_=kstage[j])

                def emit_q(j):
                    for off, w in _chunks(Pq, 1024):
                        ps = sc_tile()
                        for coff, cw in _chunks(w):
                            for k in range(KT):
                                nc.tensor.matmul(
                                    ps[:, coff:coff + cw],
                                    wq_sb[k][:, j * 128:(j + 1) * 128],
                                    xt[k][:, off + coff:off + coff + cw],
                                    start=(k == 0), stop=(k == KT - 1),
                                )
                        nc.vector.tensor_scalar_add(
                            qT[j][:, off:off + w], ps[:, 0:w], bq_t[:, j:j + 1]
                        )

                def emit_v(kc):
                    ps = sc_tile()
                    for k in range(KT):
                        nc.tensor.matmul(
                            ps[:, 0:EG], xt[k][:, kc * 128:(kc + 1) * 128],
                            wv_sb[k][:, 0:EG],
                            start=(k == 0), stop=(k == KT - 1),
                        )
                    nc.vector.tensor_copy(
                        vhat[kc][:, :, 0:D],
                        ps[:, 0:EG].rearrange("p (h d) -> p h d", h=HG),
                    )
                    nc.vector.memset(vhat[kc][:, :, D:D + 1], 1.0)
                    if want_kv:
                        nc.sync.dma_start(
                            out=out_v[kc * 128:(kc + 1) * 128, :],
                            in_=vhat[kc][:, :, 0:D],
                        )

                def emit_scores_exp(t, kc, hh, q0):
                    prows = slice(hh * D, (hh + 1) * D)
                    ps = sc_tile()
                    for off, w in _chunks(QBLK):
                        nc.tensor.matmul(
                            ps[:, off:off + w],
                            kstage[t][prows, kc * 128:(kc + 1) * 128],
                            qT[t][prows, q0 + off:q0 + off + w],
                            start=True, stop=True,
                            tile_position=(hh * D, 0),
                        )
                    ph = p2s.tile([128, QBLK], BF16, name="ph", tag="ph", bufs=4)
                    nc.scalar.activation(
                        ph, ps, mybir.ActivationFunctionType.Exp,
                        bias=nm_t[:, kc:kc + 1], scale=SCALE,
                    )
                    return ph

                def emit_av(t, kc, hh, ph, ps_av):
                    h = 2 * t + hh
                    for off, w in _chunks(QBLK):
                        nc.tensor.matmul(
                            ps_av[hh][:, off:off + w],
                            vhat[kc][:, h, :],
                            ph[:, off:off + w],
                            start=(kc == 0), stop=(kc == NKC - 1),
                        )

                def emit_evac(t, ps_av, q0, last=False):
                    """Evacuate PSUM fast: all four copies first (frees the av
                    accumulator banks for the next pair), then the
                    reciprocals (fast-approx DVE, ~18 significant bits)."""
                    lls = []
                    for hh in range(2):
                        nc.vector.tensor_copy(
                            ao[t][hh * D:(hh + 1) * D, q0:q0 + QBLK],
                            ps_av[hh][0:D, :],
                        )
                        lsb = p2s.tile([1, QBLK], F32, name="lsb", tag="lsb", bufs=2)
                        nc.vector.tensor_copy(lsb, ps_av[hh][D:D + 1, :])
                        lls.append(lsb)
                    rcps = []
                    for hh in range(2):
                        rcp = p2s.tile([1, QBLK], F32, name="rcp", tag="rcp", bufs=2)
                        nc.vector.reciprocal_approx_fast(rcp, lls[hh])
                        rbf = p2s.tile([1, QBLK], BF16, name="rbf", tag="rbf", bufs=2)
                        nc.vector.tensor_copy(rbf, rcp)
                        rcps.append(rbf)
                    return rcps

                def emit_norm_apply(t, rcps, q0):
                    """ps_b = broadcast(1/l) via ones-matmul; ao *= ps_b."""
                    ps_b = sc_tile()
                    for hh in range(2):
                        for off, w in _chunks(QBLK):
                            nc.tensor.matmul(
                                ps_b[hh * D:(hh + 1) * D, off:off + w],
                                ones_t[:, 0:D], rcps[hh][:, off:off + w],
                                start=True, stop=True,
                            )
                    dst = ao[t][:, q0:q0 + QBLK]
                    nc.vector.tensor_mul(dst, dst, ps_b)

                def emit_wo_partial(qt, q0):
                    """wops[qt] = sum over pairs 0..2 of ao[t][:, qt] @ wo[t]."""
                    ps = sc_tile()
                    for fc in range(2):
                        for t in range(PAIRS - 1):
                            nc.tensor.matmul(
                                ps[:, fc * 512:(fc + 1) * 512],
                                ao[t][:, q0 + qt * 128:q0 + (qt + 1) * 128],
                                wo_sb[t][:, fc * 512:(fc + 1) * 512],
                                start=(t == 0), stop=(t == PAIRS - 2),
                            )
                    nc.vector.tensor_copy(wops[qt], ps)

                def emit_wo_final(qt, q0, split=False):
                    """out rows qt = wops[qt] + pair 3's contribution."""
                    t = PAIRS - 1
                    ps = sc_tile()
                    for fc in range(2):
                        nc.tensor.matmul(
                            ps[:, fc * 512:(fc + 1) * 512],
                            ao[t][:, q0 + qt * 128:q0 + (qt + 1) * 128],
                            wo_sb[t][:, fc * 512:(fc + 1) * 512],
                            start=True, stop=True,
                        )
                    osb = p2s.tile([128, E], BF16, name="osb", tag="osb", bufs=3)
                    nc.vector.tensor_add(osb, wops[qt], ps)
                    nc.sync.dma_start(
                        out=out[q0 + qt * 128:q0 + (qt + 1) * 128, :], in_=osb)

                def emit_k_chunk(j, off, w):
                    ps = sc_tile()
                    for coff, cw in _chunks(w):
                        for k in range(KT):
                            nc.tensor.matmul(
                                ps[:, coff:coff + cw],
                                wk_sb[k][:, j * 128:(j + 1) * 128],
                                xt[k][:, off + coff:off + coff + cw],
                                start=(k == 0), stop=(k == KT - 1),
                            )
                    nc.vector.tensor_copy(kstage[j][:, off:off + w], ps[:, 0:w])

                # ---- emission schedule ----
                emit_k_chunk(0, 0, min(Pk, 1024))
                emit_q(0)

                # pre[(bi, t, kc)] -> deferred production tasks
                pre = {}
                if Pk > 1024:
                    pre.setdefault((0, 0, 0), []).append(("ktail", 0))
                elif want_kv:
                    nc.sync.dma_start(out=out_k[0:128, :], in_=kstage[0])
                for kc in range(2):
                    pre.setdefault((0, 0, kc), []).append(("v", kc))
                for kc in range(2, NKC):
                    pre.setdefault((0, 0, kc - 2), []).append(("v", kc))
                for t in range(1, PAIRS):
                    pre.setdefault((0, t - 1, 3), []).append(("k", t))
                    pre.setdefault((0, t - 1, 6), []).append(("q", t))

                def run_pre(tasks):
                    for kind, arg in tasks:
                        if kind == "ktail":
                            for off, w in _chunks(Pk, 1024)[1:]:
                                emit_k_chunk(arg, off, w)
                            if want_kv:
                                nc.sync.dma_start(
                                    out=out_k[arg * 128:(arg + 1) * 128, :],
                                    in_=kstage[arg])
                        elif kind == "k":
                            emit_k(arg)
                        elif kind == "q":
                            emit_q(arg)
                        elif kind == "wo":
                            for k in range(NJ):
                                nc.sync.dma_start(
                                    out=wo_sb[k], in_=wo[k * 128:(k + 1) * 128, :])
                        else:
                            emit_v(arg)

                # deferred post-pair tasks (norm_apply, WO partials/finals);
                # consumed two per kc slot during subsequent pairs.
                deferred = []

                def drain(nmax):
                    for _ in range(nmax):
                        if not deferred:
                            return
                        deferred.pop(0)()

                for bi in range(NB):
                    q0 = bi * QBLK
                    if bi == 0:
                        pre.setdefault((0, 0, 5), []).append(("wo", None))
                    for t in range(PAIRS):
                        ps_av = {
                            hh: psA.tile([D + 1, QBLK], F32, name="ps_av",
                                         tag="av", bufs=2)
                            for hh in range(2)
                        }
                        pend = []
                        for kc in range(NKC):
                            run_pre(pre.get((bi, t, kc), ()))
                            if kc >= 3 and not (bi == 0 and t == 0):
                                drain(2)
                            for hh in range(2):
                                ph = emit_scores_exp(t, kc, hh, q0)
                                pend.append((kc, hh, ph))
                                if len(pend) > 2:
                                    kc0, hh0, ph0 = pend.pop(0)
                                    emit_av(t, kc0, hh0, ph0, ps_av)
                        for kc0, hh0, ph0 in pend:
                            emit_av(t, kc0, hh0, ph0, ps_av)
                        last = (bi == NB - 1) and (t == PAIRS - 1)
                        rcps = emit_evac(t, ps_av, q0, last=last)
                        deferred.append(
                            lambda t=t, r=rcps, q=q0: emit_norm_apply(t, r, q))
                        if t == 2:
                            deferred.extend(
                                lambda qt=qt, q=q0: emit_wo_partial(qt, q)
                                for qt in range(QBLK // 128))
                        elif t == PAIRS - 1:
                            tail = bi == NB - 1
                            deferred.extend(
                                lambda qt=qt, q=q0, s=tail: emit_wo_final(qt, q, s)
                                for qt in range(QBLK // 128))
                for f in deferred:
                    f()
    nc.compile()
    return nc


def _make_executor(key):
    """Build the Bass program once and wrap it in a cached jitted shard_map."""
    import jax
    from jax.experimental.shard_map import shard_map
    from jax.sharding import Mesh, PartitionSpec, NamedSharding
    from concourse.bass2jax import (
        _bass_exec_p,
        install_neuronx_cc_hook,
        partition_id_tensor,
    )

    nc = _build_program(*key)
    install_neuronx_cc_hook()
    assert nc.dbg_addr is None
    partition_name = nc.partition_id_tensor.name if nc.partition_id_tensor else None

    in_names, out_names, out_avals, zero_outs = [], [], [], []
    for alloc in nc.m.functions[0].allocations:
        if not isinstance(alloc, mybir.MemoryLocationSet):
            continue
        name = alloc.memorylocations[0].name
        if alloc.kind == "ExternalInput":
            if name != partition_name:
                in_names.append(name)
        elif alloc.kind == "ExternalOutput":
            shape = tuple(alloc.tensor_shape)
            dtype = mybir.dt.np(alloc.dtype)
            out_names.append(name)
            out_avals.append(jax.core.ShapedArray(shape, dtype))
            zero_outs.append(np.zeros(shape, dtype))
    n_params = len(in_names)
    n_outs = len(out_avals)
    all_names = in_names + out_names
    if partition_name is not None:
        all_names = all_names + [partition_name]
    donate = tuple(range(n_params, n_params + n_outs))

    def _body(*args):
        operands = list(args)
        if partition_name is not None:
            operands.append(partition_id_tensor())
        outs = _bass_exec_p.bind(
            *operands,
            out_avals=tuple(out_avals),
            in_names=tuple(all_names),
            out_names=tuple(out_names),
            lowering_input_output_aliases=(),
            sim_require_finite=True,
            sim_require_nnan=True,
            nc=nc,
        )
        return tuple(outs)

    devices = jax.devices()[:N_CORES]
    mesh = Mesh(np.asarray(devices), ("core",))
    in_specs = (PartitionSpec("core"),) * (n_params + n_outs)
    out_specs = (PartitionSpec("core"),) * n_outs
    sharded = jax.jit(
        shard_map(_body, mesh=mesh, in_specs=in_specs, out_specs=out_specs,
                  check_rep=False),
        donate_argnums=donate,
        keep_unused=True,
    )
    sharding = NamedSharding(mesh, PartitionSpec("core"))
    return {
        "jit": sharded, "in_names": in_names, "out_names": out_names,
        "out_avals": out_avals, "zero_outs": zero_outs, "sharding": sharding,
        "jax": jax, "nc": nc,
    }


def get_executor(key):
    ck = ("ex",) + key
    if ck not in _prog_cache:
        _prog_cache[ck] = _make_executor(key)
    return _prog_cache[ck]


def run_spmd(in_maps, key):
    ex = get_executor(key)
    jax = ex["jax"]
    concat_in = [
        np.concatenate([np.asarray(m[name]) for m in in_maps], axis=0)
        for name in ex["in_names"]
    ]
    concat_zeros = [
        np.zeros((N_CORES * z.shape[0], *z.shape[1:]), z.dtype)
        for z in ex["zero_outs"]
    ]
    out_arrs = ex["jit"](*concat_in, *concat_zeros)
    return [
        {
            name: np.asarray(out_arrs[i]).reshape(N_CORES, *ex["out_avals"][i].shape)[c]
            for i, name in enumerate(ex["out_names"])
        }
        for c in range(N_CORES)
    ]


def plan_shapes(mask):
    """Returns (Pk, Pq, want_kv, idxs)."""
    idxs = [np.nonzero(mask[b])[0] for b in range(B)]
    max_m = max((len(ix) for ix in idxs), default=0)
    max_m = max(max_m, 1)
    Pk = ((max_m + 127) // 128) * 128
    nb = max(1, -(-(max_m - OVF) // QBLK))
    Pq = nb * QBLK
    Pk = max(Pk, Pq)   # queries are a prefix of the key rows
    want_kv = max_m > Pq
    return Pk, Pq, want_kv, idxs


def build_in_maps(x, mask, WQ_w, WQ_b, WK_w, WK_b, WV_w, WV_b, WO_w, WO_b):
    x = np.asarray(x, dtype=np.float32)
    mask = np.asarray(mask).astype(bool)
    WQ_w = np.asarray(WQ_w, dtype=np.float32)
    WQ_b = np.asarray(WQ_b, dtype=np.float32)
    WK_w = np.asarray(WK_w, dtype=np.float32)
    WV_w = np.asarray(WV_w, dtype=np.float32)
    WO_w = np.asarray(WO_w, dtype=np.float32)

    Pk, Pq, want_kv, idxs = plan_shapes(mask)
    NKC = Pk // 128

    wq_t = np.ascontiguousarray(WQ_w.T).astype(ml_dtypes.bfloat16)
    wk_t = np.ascontiguousarray(WK_w.T).astype(ml_dtypes.bfloat16)
    wv_t = np.ascontiguousarray(WV_w.T).astype(ml_dtypes.bfloat16)
    wo_t = np.ascontiguousarray(WO_w.T).astype(ml_dtypes.bfloat16)

    in_maps = []
    for c in range(N_CORES):
        b, hg = divmod(c, 2)
        ix = idxs[b]
        m = len(ix)
        xv = np.zeros((E, Pk), dtype=ml_dtypes.bfloat16)
        xv[:, :m] = x[b, ix].T
        cols = slice(hg * EG, (hg + 1) * EG)
        nm = np.full(Pk, -1e6, dtype=np.float32)
        nm[:m] = 0.0
        bq_sl = WQ_b[cols].reshape(NJ, 128).T.astype(np.float32)
        in_maps.append({
            "xT": xv,
            "wq": np.ascontiguousarray(wq_t[:, cols]),
            "wk": np.ascontiguousarray(wk_t[:, cols]),
            "wv": np.ascontiguousarray(wv_t[:, cols]),
            "wo": np.ascontiguousarray(wo_t[cols, :]),
            "bq": np.ascontiguousarray(bq_sl),
            "negmask": np.ascontiguousarray(nm.reshape(NKC, 128).T),
        })
    return in_maps, (Pk, Pq, want_kv), idxs


def combine(results, key, idxs, x, WQ_w, WQ_b, WV_b, WO_w, WO_b):
    """Host-side unshard: sum partials, finish overflow rows, bias+abs+scatter."""
    Pk, Pq, want_kv = key
    bo_eff = (np.asarray(WO_w, np.float32) @ np.asarray(WV_b, np.float32)
              + np.asarray(WO_b, np.float32))
    out = np.zeros((B, S, E), dtype=np.float32)
    for b in range(B):
        ix = idxs[b]
        m = len(ix)
        md = min(m, Pq)
        part = (results[2 * b]["out"][:md].astype(np.float32)
                + results[2 * b + 1]["out"][:md].astype(np.float32)
                + bo_eff[None, :])
        out[b, ix[:md], :] = np.abs(part)
        if m > Pq:
            # overflow rows: full attention in numpy using device K^T / V
            r = np.asarray(x[b], np.float32)[ix[Pq:]]              # [rq, E]
            q = (r @ np.asarray(WQ_w, np.float32).T
                 + np.asarray(WQ_b, np.float32)).reshape(-1, H, D)  # [rq, H, D]
            kT = np.concatenate(
                [results[2 * b]["out_k"].astype(np.float32)[:, :m],
                 results[2 * b + 1]["out_k"].astype(np.float32)[:, :m]],
                axis=0,
            ).reshape(H, D, m)                                       # [H, D, m]
            v = np.concatenate(
                [results[2 * b]["out_v"].astype(np.float32)[:m],
                 results[2 * b + 1]["out_v"].astype(np.float32)[:m]],
                axis=1,
            ).reshape(m, H, D)                                       # [m, H, D]
            sc = np.einsum("qhd,hdm->qhm", q, kT) * SCALE
            p = np.exp(sc)
            o = np.einsum("qhm,mhd->qhd", p, v) / p.sum(-1)[..., None]
            o = o.reshape(-1, H * D) @ np.asarray(WO_w, np.float32).T + bo_eff
            out[b, ix[Pq:], :] = np.abs(o)
    return out


def kernel(x, mask, WQ_w, WQ_b, WK_w, WK_b, WV_w, WV_b, WO_w, WO_b):
    mask = np.asarray(mask).astype(bool)
    in_maps, key, idxs = build_in_maps(
        x, mask, WQ_w, WQ_b, WK_w, WK_b, WV_w, WV_b, WO_w, WO_b)
    results = run_spmd(in_maps, key)
    return combine(results, key, idxs, x, WQ_w, WQ_b, WV_b, WO_w, WO_b)


# revision 20
# speedup vs baseline: 347.2371x; 1.0067x over previous
"""Multi-head self-attention (B=4, S=2048, E=1024, H=16) on 8 TRN2 NeuronCores.

v3 strategy: mask compaction + head parallelism, no collectives.

Observation: the reference zeroes output rows where mask==0 (o * mask before
abs), and masked keys get -1e6 scores (zero softmax weight). With a ~50%
random mask, half the rows are dead. The host compacts each batch to its
valid rows and scatters results back.

Sharding: core c handles batch b=c//2 and head-group hg=c%2 (8 of 16 heads).
Each core computes K/V for all Pk compacted rows and Q for the first Pq
(=1024*nb) compacted rows of its batch, for its 8 heads (512-feature weight
slices), runs full attention, and emits the partial output projection
o_part = attn_out @ WO[:, hg]^T in bf16. The host sums the two partials per
batch, adds bo_eff = WO@bV + bO, applies abs, and scatters into the zeroed
full output. The <=64 query rows beyond Pq ("overflow") are finished on the
host in numpy, using K^T and V fetched from the device (tiny extra DMA).

Math notes (exactness-preserving rewrites):
- K bias dropped (softmax invariant); V bias folded into host-side bo_eff.
- 1/sqrt(D) and the pad-key mask (-1e6) fused into the Exp activation.
- No max-subtraction: scores are O(1), exp cannot overflow.
- Softmax denominator l rides as a ones-column in V-hat. Per pair, the raw
  attention output and l are evacuated from PSUM with two cheap copies
  (freeing the accumulator banks immediately); the reciprocal + ones-matmul
  broadcast + multiply then run off the critical path.

PSUM (8 banks): scores/projections/WO share tag 'sc' [128,1024] f32 x2 bufs
(4 banks); av accumulators [65,1024] f32 x2 (4 banks). Query blocks are
exactly 1024 wide, so each Exp activation is one (1024+352)-cycle ACTIVATE.
"""

import sys

if "/opt/trn_rl_repo" not in sys.path:
    sys.path.insert(0, "/opt/trn_rl_repo")

import numpy as np
import ml_dtypes

import concourse.bass as bass
import concourse.mybir as mybir
from concourse import bacc
from concourse.tile import TileContext

BF16 = mybir.dt.bfloat16
F32 = mybir.dt.float32

B, S, E, H = 4, 2048, 1024, 16
D = E // H          # 64
N_CORES = 8
HG = H // 2         # 8 heads per core
EG = HG * D         # 512 head-group features per core
KT = E // 128       # 8 contraction tiles over E
NJ = EG // 128      # 4 feature tiles (j) per head group
PAIRS = NJ          # 4 head pairs per core
SCALE = 1.0 / 8.0   # 1/sqrt(D)
QBLK = 1024         # query block width (PSUM-sized)
OVF = 64            # max query rows finished on the host

_prog_cache = {}


def _chunks(n, step=512):
    out, off = [], 0
    while off < n:
        w = min(step, n - off)
        out.append((off, w))
        off += w
    return out


def _build_program(Pk, Pq, want_kv):
    NKC = Pk // 128      # key chunks
    NB = Pq // QBLK      # query blocks
    nc = bacc.Bacc("TRN2", target_bir_lowering=False, debug=False, num_devices=N_CORES)

    xT = nc.dram_tensor("xT", [E, Pk], BF16, kind="ExternalInput").ap()
    wq = nc.dram_tensor("wq", [E, EG], BF16, kind="ExternalInput").ap()
    wk = nc.dram_tensor("wk", [E, EG], BF16, kind="ExternalInput").ap()
    wv = nc.dram_tensor("wv", [E, EG], BF16, kind="ExternalInput").ap()
    wo = nc.dram_tensor("wo", [EG, E], BF16, kind="ExternalInput").ap()
    bq = nc.dram_tensor("bq", [128, NJ], F32, kind="ExternalInput").ap()
    negmask = nc.dram_tensor("negmask", [128, NKC], F32, kind="ExternalInput").ap()
    out = nc.dram_tensor("out", [Pq, E], BF16, kind="ExternalOutput").ap()
    if want_kv:
        out_k = nc.dram_tensor("out_k", [EG, Pk], BF16, kind="ExternalOutput").ap()
        out_v = nc.dram_tensor("out_v", [Pk, EG], BF16, kind="ExternalOutput").ap()

    with TileContext(nc) as tc:
        with tc.tile_pool(name="persist", bufs=1) as persist:
            bq_t = persist.tile([128, NJ], F32)
            nc.sync.dma_start(out=bq_t, in_=bq[:, :])
            nm_t = persist.tile([128, NKC], F32)
            nc.sync.dma_start(out=nm_t, in_=negmask[:, :])
            ones_t = persist.tile([1, 128], BF16)
            nc.vector.memset(ones_t, 1.0)
            ones32 = persist.tile([1, 128], F32)
            nc.vector.memset(ones32, 1.0)

            xt = [persist.tile([128, Pk], BF16, name=f"xt{k}") for k in range(KT)]
            wk_sb = [persist.tile([128, EG], BF16, name=f"wk{k}") for k in range(KT)]
            wq_sb = [persist.tile([128, EG], BF16, name=f"wq{k}") for k in range(KT)]
            wv_sb = [persist.tile([128, EG], BF16, name=f"wv{k}") for k in range(KT)]
            wo_sb = [persist.tile([128, E], BF16, name=f"wo{k}") for k in range(NJ)]
            kstage = [persist.tile([128, Pk], BF16, name=f"kst{j}") for j in range(NJ)]
            qT = [persist.tile([128, Pq], BF16, name=f"qT{j}") for j in range(NJ)]
            vhat = [persist.tile([128, HG, D + 1], BF16, name=f"vh{v}") for v in range(NKC)]
            ao = [persist.tile([128, Pq], BF16, name=f"ao{t}") for t in range(PAIRS)]
            wops = [persist.tile([128, E], BF16, name=f"wop{q}")
                    for q in range(QBLK // 128)]

            for k in range(KT):
                nc.sync.dma_start(out=xt[k][:, 0:512],
                                  in_=xT[k * 128:(k + 1) * 128, 0:512])
                nc.sync.dma_start(out=wk_sb[k][:, 0:128],
                                  in_=wk[k * 128:(k + 1) * 128, 0:128])
                nc.sync.dma_start(out=wq_sb[k][:, 0:128],
                                  in_=wq[k * 128:(k + 1) * 128, 0:128])
            for k in range(KT):
                nc.sync.dma_start(out=xt[k][:, 512:Pk],
                                  in_=xT[k * 128:(k + 1) * 128, 512:Pk])
            for k in range(KT):
                nc.sync.dma_start(out=wv_sb[k], in_=wv[k * 128:(k + 1) * 128, :])
            for k in range(KT):
                nc.sync.dma_start(out=wk_sb[k][:, 128:EG],
                                  in_=wk[k * 128:(k + 1) * 128, 128:EG])
                nc.sync.dma_start(out=wq_sb[k][:, 128:EG],
                                  in_=wq[k * 128:(k + 1) * 128, 128:EG])

            with (
                tc.tile_pool(name="p2s", bufs=3) as p2s,
                tc.tile_pool(name="psA", bufs=1, space="PSUM") as psA,
            ):
                def sc_tile():
                    return psA.tile([128, 1024], F32, name="ps_sc", tag="sc", bufs=2)

                def emit_k(j):
                    for off, w in _chunks(Pk, 1024):
                        ps = sc_tile()
                        for coff, cw in _chunks(w):
                            for k in range(KT):
                                nc.tensor.matmul(
                                    ps[:, coff:coff + cw],
                                    wk_sb[k][:, j * 128:(j + 1) * 128],
                                    xt[k][:, off + coff:off + coff + cw],
                                    start=(k == 0), stop=(k == KT - 1),
                                )
                        nc.vector.tensor_copy(kstage[j][:, off:off + w], ps[:, 0:w])
                    if want_kv:
                        nc.sync.dma_start(out=out_k[j * 128:(j + 1) * 128, :],
                                          in_=kstage[j])

                def emit_q(j):
                    for off, w in _chunks(Pq, 1024):
                        ps = sc_tile()
                        for coff, cw in _chunks(w):
                            for k in range(KT):
                                nc.tensor.matmul(
                                    ps[:, coff:coff + cw],
                                    wq_sb[k][:, j * 128:(j + 1) * 128],
                                    xt[k][:, off + coff:off + coff + cw],
                                    start=(k == 0), stop=(k == KT - 1),
                                )
                        nc.vector.tensor_scalar_add(
                            qT[j][:, off:off + w], ps[:, 0:w], bq_t[:, j:j + 1]
                        )

                def emit_v(kc):
                    ps = sc_tile()
                    for k in range(KT):
                        nc.tensor.matmul(
                            ps[:, 0:EG], xt[k][:, kc * 128:(kc + 1) * 128],
                            wv_sb[k][:, 0:EG],
                            start=(k == 0), stop=(k == KT - 1),
                        )
                    nc.vector.tensor_copy(
                        vhat[kc][:, :, 0:D],
                        ps[:, 0:EG].rearrange("p (h d) -> p h d", h=HG),
                    )
                    nc.vector.memset(vhat[kc][:, :, D:D + 1], 1.0)
                    if want_kv:
                        nc.sync.dma_start(
                            out=out_v[kc * 128:(kc + 1) * 128, :],
                            in_=vhat[kc][:, :, 0:D],
                        )

                def emit_scores_exp(t, kc, hh, q0):
                    prows = slice(hh * D, (hh + 1) * D)
                    ps = sc_tile()
                    for off, w in _chunks(QBLK):
                        nc.tensor.matmul(
                            ps[:, off:off + w],
                            kstage[t][prows, kc * 128:(kc + 1) * 128],
                            qT[t][prows, q0 + off:q0 + off + w],
                            start=True, stop=True,
                            tile_position=(hh * D, 0),
                        )
                    ph = p2s.tile([128, QBLK], BF16, name="ph", tag="ph", bufs=4)
                    nc.scalar.activation(
                        ph, ps, mybir.ActivationFunctionType.Exp,
                        bias=nm_t[:, kc:kc + 1], scale=SCALE,
                    )
                    return ph

                def emit_av(t, kc, hh, ph, ps_av):
                    h = 2 * t + hh
                    for off, w in _chunks(QBLK):
                        nc.tensor.matmul(
                            ps_av[hh][:, off:off + w],
                            vhat[kc][:, h, :],
                            ph[:, off:off + w],
                            start=(kc == 0), stop=(kc == NKC - 1),
                        )

                def emit_evac(t, ps_av, q0, last=False):
                    """Evacuate PSUM fast: all four copies first (frees the av
                    accumulator banks for the next pair), then the
                    reciprocals (fast-approx DVE, ~18 significant bits)."""
                    lls = []
                    for hh in range(2):
                        nc.vector.tensor_copy(
                            ao[t][hh * D:(hh + 1) * D, q0:q0 + QBLK],
                            ps_av[hh][0:D, :],
                        )
                        lsb = p2s.tile([1, QBLK], F32, name="lsb", tag="lsb", bufs=2)
                        nc.vector.tensor_copy(lsb, ps_av[hh][D:D + 1, :])
                        lls.append(lsb)
                    rcps = []
                    for hh in range(2):
                        rcp = p2s.tile([1, QBLK], F32, name="rcp", tag="rcp", bufs=2)
                        nc.vector.reciprocal_approx_fast(rcp, lls[hh])
                        rbf = p2s.tile([1, QBLK], BF16, name="rbf", tag="rbf", bufs=2)
                        nc.vector.tensor_copy(rbf, rcp)
                        rcps.append(rbf)
                    return rcps

                def emit_norm_apply(t, rcps, q0):
                    """ps_b = broadcast(1/l) via ones-matmul; ao *= ps_b."""
                    ps_b = sc_tile()
                    for hh in range(2):
                        for off, w in _chunks(QBLK):
                            nc.tensor.matmul(
                                ps_b[hh * D:(hh + 1) * D, off:off + w],
                                ones_t[:, 0:D], rcps[hh][:, off:off + w],
                                start=True, stop=True,
                            )
                    dst = ao[t][:, q0:q0 + QBLK]
                    nc.vector.tensor_mul(dst, dst, ps_b)

                def emit_wo_partial(qt, q0):
                    """wops[qt] = sum over pairs 0..2 of ao[t][:, qt] @ wo[t]."""
                    ps = sc_tile()
                    for fc in range(2):
                        for t in range(PAIRS - 1):
                            nc.tensor.matmul(
                                ps[:, fc * 512:(fc + 1) * 512],
                                ao[t][:, q0 + qt * 128:q0 + (qt + 1) * 128],
                                wo_sb[t][:, fc * 512:(fc + 1) * 512],
                                start=(t == 0), stop=(t == PAIRS - 2),
                            )
                    nc.vector.tensor_copy(wops[qt], ps)

                def emit_wo_final(qt, q0, split=False):
                    """out rows qt = wops[qt] + pair 3's contribution."""
                    t = PAIRS - 1
                    ps = sc_tile()
                    for fc in range(2):
                        nc.tensor.matmul(
                            ps[:, fc * 512:(fc + 1) * 512],
                            ao[t][:, q0 + qt * 128:q0 + (qt + 1) * 128],
                            wo_sb[t][:, fc * 512:(fc + 1) * 512],
                            start=True, stop=True,
                        )
                    osb = p2s.tile([128, E], BF16, name="osb", tag="osb", bufs=3)
                    nc.vector.tensor_add(osb, wops[qt], ps)
                    nc.sync.dma_start(
                        out=out[q0 + qt * 128:q0 + (qt + 1) * 128, :], in_=osb)

                def emit_k_chunk(j, off, w):
                    ps = sc_tile()
                    for coff, cw in _chunks(w):
                        for k in range(KT):
                            nc.tensor.matmul(
                                ps[:, coff:coff + cw],
                                wk_sb[k][:, j * 128:(j + 1) * 128],
                                xt[k][:, off + coff:off + coff + cw],
                                start=(k == 0), stop=(k == KT - 1),
                            )
                    nc.vector.tensor_copy(kstage[j][:, off:off + w], ps[:, 0:w])

                # ---- emission schedule ----
                emit_k_chunk(0, 0, min(Pk, 1024))
                emit_q(0)

                # pre[(bi, t, kc)] -> deferred production tasks
                pre = {}
                if Pk > 1024:
                    pre.setdefault((0, 0, 0), []).append(("ktail", 0))
                elif want_kv:
                    nc.sync.dma_start(out=out_k[0:128, :], in_=kstage[0])
                for kc in range(2):
                    pre.setdefault((0, 0, kc), []).append(("v", kc))
                for kc in range(2, NKC):
                    pre.setdefault((0, 0, kc - 2), []).append(("v", kc))
                for t in range(1, PAIRS):
                    pre.setdefault((0, t - 1, 3), []).append(("k", t))
                    pre.setdefault((0, t - 1, 6), []).append(("q", t))

                def run_pre(tasks):
                    for kind, arg in tasks:
                        if kind == "ktail":
                            for off, w in _chunks(Pk, 1024)[1:]:
                                emit_k_chunk(arg, off, w)
                            if want_kv:
                                nc.sync.dma_start(
                                    out=out_k[arg * 128:(arg + 1) * 128, :],
                                    in_=kstage[arg])
                        elif kind == "k":
                            emit_k(arg)
                        elif kind == "q":
                            emit_q(arg)
                        elif kind == "wo":
                            for k in range(NJ):
                                nc.sync.dma_start(
                                    out=wo_sb[k], in_=wo[k * 128:(k + 1) * 128, :])
                        else:
                            emit_v(arg)

                # deferred post-pair tasks (norm_apply, WO partials/finals);
                # consumed two per kc slot during subsequent pairs.
                deferred = []

                def drain(nmax):
                    for _ in range(nmax):
                        if not deferred:
                            return
                        deferred.pop(0)()

                for bi in range(NB):
                    q0 = bi * QBLK
                    if bi == 0:
                        pre.setdefault((0, 0, 5), []).append(("wo", None))
                    for t in range(PAIRS):
                        ps_av = {
                            hh: psA.tile([D + 1, QBLK], F32, name="ps_av",
                                         tag="av", bufs=2)
                            for hh in range(2)
                        }
                        pend = []
                        for kc in range(NKC):
                            run_pre(pre.get((bi, t, kc), ()))
                            if kc >= 3 and not (bi == 0 and t == 0):
                                drain(2)
                            for hh in range(2):
                                ph = emit_scores_exp(t, kc, hh, q0)
                                pend.append((kc, hh, ph))
                                if len(pend) > 2:
                                    kc0, hh0, ph0 = pend.pop(0)
                                    emit_av(t, kc0, hh0, ph0, ps_av)
                        for kc0, hh0, ph0 in pend:
                            emit_av(t, kc0, hh0, ph0, ps_av)
                        last = (bi == NB - 1) and (t == PAIRS - 1)
                        rcps = emit_evac(t, ps_av, q0, last=last)
                        deferred.append(
                            lambda t=t, r=rcps, q=q0: emit_norm_apply(t, r, q))
                        if t == 2:
                            deferred.extend(
                                lambda qt=qt, q=q0: emit_wo_partial(qt, q)
                                for qt in range(QBLK // 128))
                        elif t == PAIRS - 1:
                            tail = bi == NB - 1
                            deferred.extend(
                                lambda qt=qt, q=q0, s=tail: emit_wo_final(qt, q, s)
                                for qt in range(QBLK // 128))
                for f in deferred:
                    f()
    nc.compile()
    return nc


def _make_executor(key):
    """Build the Bass program once and wrap it in a cached jitted shard_map."""
    import jax
    from jax.experimental.shard_map import shard_map
    from jax.sharding import Mesh, PartitionSpec, NamedSharding
    from concourse.bass2jax import (
        _bass_exec_p,
        install_neuronx_cc_hook,
        partition_id_tensor,
    )

    nc = _build_program(*key)
    install_neuronx_cc_hook()
    assert nc.dbg_addr is None
    partition_name = nc.partition_id_tensor.name if nc.partition_id_tensor else None

    in_names, out_names, out_avals, zero_outs = [], [], [], []
    for alloc in nc.m.functions[0].allocations:
        if not isinstance(alloc, mybir.MemoryLocationSet):
            continue
        name = alloc.memorylocations[0].name
        if alloc.kind == "ExternalInput":
            if name != partition_name:
                in_names.append(name)
        elif alloc.kind == "ExternalOutput":
            shape = tuple(alloc.tensor_shape)
            dtype = mybir.dt.np(alloc.dtype)
            out_names.append(name)
            out_avals.append(jax.core.ShapedArray(shape, dtype))
            zero_outs.append(np.zeros(shape, dtype))
    n_params = len(in_names)
    n_outs = len(out_avals)
    all_names = in_names + out_names
    if partition_name is not None:
        all_names = all_names + [partition_name]
    donate = tuple(range(n_params, n_params + n_outs))

    def _body(*args):
        operands = list(args)
        if partition_name is not None:
            operands.append(partition_id_tensor())
        outs = _bass_exec_p.bind(
            *operands,
            out_avals=tuple(out_avals),
            in_names=tuple(all_names),
            out_names=tuple(out_names),
            lowering_input_output_aliases=(),
            sim_require_finite=True,
            sim_require_nnan=True,
            nc=nc,
        )
        return tuple(outs)

    devices = jax.devices()[:N_CORES]
    mesh = Mesh(np.asarray(devices), ("core",))
    in_specs = (PartitionSpec("core"),) * (n_params + n_outs)
    out_specs = (PartitionSpec("core"),) * n_outs
    sharded = jax.jit(
        shard_map(_body, mesh=mesh, in_specs=in_specs, out_specs=out_specs,
                  check_rep=False),
        donate_argnums=donate,
        keep_unused=True,
    )
    sharding = NamedSharding(mesh, PartitionSpec("core"))
    return {
        "jit": sharded, "in_names": in_names, "out_names": out_names,
        "out_avals": out_avals, "zero_outs": zero_outs, "sharding": sharding,
        "jax": jax, "nc": nc,
    }


def get_executor(key):
    ck = ("ex",) + key
    if ck not in _prog_cache:
        _prog_cache[ck] = _make_executor(key)
    return _prog_cache[ck]


def run_spmd(in_maps, key):
    ex = get_executor(key)
    jax = ex["jax"]
    concat_in = [
        np.concatenate([np.asarray(m[name]) for m in in_maps], axis=0)
        for name in ex["in_names"]
    ]
    concat_zeros = [
        np.zeros((N_CORES * z.shape[0], *z.shape[1:]), z.dtype)
        for z in ex["zero_outs"]
    ]
    out_arrs = ex["jit"](*concat_in, *concat_zeros)
    return [
        {
            name: np.asarray(out_arrs[i]).reshape(N_CORES, *ex["out_avals"][i].shape)[c]
            for i, name in enumerate(ex["out_names"])
        }
        for c in range(N_CORES)
    ]


def plan_shapes(mask):
    """Returns (Pk, Pq, want_kv, idxs)."""
    idxs = [np.nonzero(mask[b])[0] for b in range(B)]
    max_m = max((len(ix) for ix in idxs), default=0)
    max_m = max(max_m, 1)
    Pk = ((max_m + 127) // 128) * 128
    nb = max(1, -(-(max_m - OVF) // QBLK))
    Pq = nb * QBLK
    Pk = max(Pk, Pq)   # queries are a prefix of the key rows
    want_kv = max_m > Pq
    return Pk, Pq, want_kv, idxs


def build_in_maps(x, mask, WQ_w, WQ_b, WK_w, WK_b, WV_w, WV_b, WO_w, WO_b):
    x = np.asarray(x, dtype=np.float32)
    mask = np.asarray(mask).astype(bool)
    WQ_w = np.asarray(WQ_w, dtype=np.float32)
    WQ_b = np.asarray(WQ_b, dtype=np.float32)
    WK_w = np.asarray(WK_w, dtype=np.float32)
    WV_w = np.asarray(WV_w, dtype=np.float32)
    WO_w = np.asarray(WO_w, dtype=np.float32)

    Pk, Pq, want_kv, idxs = plan_shapes(mask)
    NKC = Pk // 128

    wq_t = np.ascontiguousarray(WQ_w.T).astype(ml_dtypes.bfloat16)
    wk_t = np.ascontiguousarray(WK_w.T).astype(ml_dtypes.bfloat16)
    wv_t = np.ascontiguousarray(WV_w.T).astype(ml_dtypes.bfloat16)
    wo_t = np.ascontiguousarray(WO_w.T).astype(ml_dtypes.bfloat16)

    in_maps = []
    for c in range(N_CORES):
        b, hg = divmod(c, 2)
        ix = idxs[b]
        m = len(ix)
        xv = np.zeros((E, Pk), dtype=ml_dtypes.bfloat16)
        xv[:, :m] = x[b, ix].T
        cols = slice(hg * EG, (hg + 1) * EG)
        nm = np.full(Pk, -1e6, dtype=np.float32)
        nm[:m] = 0.0
        bq_sl = WQ_b[cols].reshape(NJ, 128).T.astype(np.float32)
        in_maps.append({
            "xT": xv,
            "wq": np.ascontiguousarray(wq_t[:, cols]),
            "wk": np.ascontiguousarray(wk_t[:, cols]),
            "wv": np.ascontiguousarray(wv_t[:, cols]),
            "wo": np.ascontiguousarray(wo_t[cols, :]),
            "bq": np.ascontiguousarray(bq_sl),
            "negmask": np.ascontiguousarray(nm.reshape(NKC, 128).T),
        })
    return in_maps, (Pk, Pq, want_kv), idxs


def combine(results, key, idxs, x, WQ_w, WQ_b, WV_b, WO_w, WO_b):
    """Host-side unshard: sum partials, finish overflow rows, bias+abs+scatter."""
    Pk, Pq, want_kv = key
    bo_eff = (np.asarray(WO_w, np.float32) @ np.asarray(WV_b, np.float32)
              + np.asarray(WO_b, np.float32))
    out = np.zeros((B, S, E), dtype=np.float32)
    for b in range(B):
        ix = idxs[b]
        m = len(ix)
        md = min(m, Pq)
        part = (results[2 * b]["out"][:md].astype(np.float32)
                + results[2 * b + 1]["out"][:md].astype(np.float32)
                + bo_eff[None, :])
        out[b, ix[:md], :] = np.abs(part)
        if m > Pq:
            # overflow rows: full attention in numpy using device K^T / V
            r = np.asarray(x[b], np.float32)[ix[Pq:]]              # [rq, E]
            q = (r @ np.asarray(WQ_w, np.float32).T
                 + np.asarray(WQ_b, np.float32)).reshape(-1, H, D)  # [rq, H, D]
            kT = np.concatenate(
                [results[2 * b]["out_k"].astype(np.float32)[:, :m],
                 results[2 * b + 1]["out_k"].astype(np.float32)[:, :m]],
                axis=0,
            ).reshape(H, D, m)                                       # [H, D, m]
            v = np.concatenate(
                [results[2 * b]["out_v"].astype(np.float32)[:m],
                 results[2 * b + 1]["out_v"].astype(np.float32)[:m]],
                axis=1,
            ).reshape(m, H, D)                                       # [m, H, D]
            sc = np.einsum("qhd,hdm->qhm", q, kT) * SCALE
            p = np.exp(sc)
            o = np.einsum("qhm,mhd->qhd", p, v) / p.sum(-1)[..., None]
            o = o.reshape(-1, H * D) @ np.asarray(WO_w, np.float32).T + bo_eff
            out[b, ix[Pq:], :] = np.abs(o)
    return out


def kernel(x, mask, WQ_w, WQ_b, WK_w, WK_b, WV_w, WV_b, WO_w, WO_b):
    mask = np.asarray(mask).astype(bool)
    in_maps, key, idxs = build_in_maps(
        x, mask, WQ_w, WQ_b, WK_w, WK_b, WV_w, WV_b, WO_w, WO_b)
    results = run_spmd(in_maps, key)
    return combine(results, key, idxs, x, WQ_w, WQ_b, WV_b, WO_w, WO_b)


# revision 23
# speedup vs baseline: 356.7532x; 1.0274x over previous
"""Multi-head self-attention (B=4, S=2048, E=1024, H=16) on 8 TRN2 NeuronCores.

v3 strategy: mask compaction + head parallelism, no collectives.

Observation: the reference zeroes output rows where mask==0 (o * mask before
abs), and masked keys get -1e6 scores (zero softmax weight). With a ~50%
random mask, half the rows are dead. The host compacts each batch to its
valid rows and scatters results back.

Sharding: core c handles batch b=c//2 and head-group hg=c%2 (8 of 16 heads).
Each core computes K/V for all Pk compacted rows and Q for the first Pq
(=1024*nb) compacted rows of its batch, for its 8 heads (512-feature weight
slices), runs full attention, and emits the partial output projection
o_part = attn_out @ WO[:, hg]^T in bf16. The host sums the two partials per
batch, adds bo_eff = WO@bV + bO, applies abs, and scatters into the zeroed
full output. The <=64 query rows beyond Pq ("overflow") are finished on the
host in numpy, using K^T and V fetched from the device (tiny extra DMA).

Math notes (exactness-preserving rewrites):
- K bias dropped (softmax invariant); V bias folded into host-side bo_eff.
- 1/sqrt(D) and the pad-key mask (-1e6) fused into the Exp activation.
- No max-subtraction: scores are O(1), exp cannot overflow.
- Softmax denominator l rides as a ones-column in V-hat. Per pair, the raw
  attention output and l are evacuated from PSUM with two cheap copies
  (freeing the accumulator banks immediately); the reciprocal + ones-matmul
  broadcast + multiply then run off the critical path.

PSUM (8 banks): scores/projections/WO share tag 'sc' [128,1024] f32 x2 bufs
(4 banks); av accumulators [65,1024] f32 x2 (4 banks). Query blocks are
exactly 1024 wide, so each Exp activation is one (1024+352)-cycle ACTIVATE.
"""

import sys

if "/opt/trn_rl_repo" not in sys.path:
    sys.path.insert(0, "/opt/trn_rl_repo")

import numpy as np
import ml_dtypes

import concourse.bass as bass
import concourse.mybir as mybir
from concourse import bacc
from concourse.tile import TileContext

BF16 = mybir.dt.bfloat16
F32 = mybir.dt.float32

B, S, E, H = 4, 2048, 1024, 16
D = E // H          # 64
N_CORES = 8
HG = H // 2         # 8 heads per core
EG = HG * D         # 512 head-group features per core
KT = E // 128       # 8 contraction tiles over E
NJ = EG // 128      # 4 feature tiles (j) per head group
PAIRS = NJ          # 4 head pairs per core
SCALE = 1.0 / 8.0   # 1/sqrt(D)
QBLK = 1024         # query block width (PSUM-sized)
OVF = 64            # max query rows finished on the host

_prog_cache = {}


def _chunks(n, step=512):
    out, off = [], 0
    while off < n:
        w = min(step, n - off)
        out.append((off, w))
        off += w
    return out


def _build_program(Pk, Pq, want_kv):
    NKC = Pk // 128      # key chunks
    NB = Pq // QBLK      # query blocks
    nc = bacc.Bacc("TRN2", target_bir_lowering=False, debug=False, num_devices=N_CORES)

    xT = nc.dram_tensor("xT", [E, Pk], BF16, kind="ExternalInput").ap()
    wq = nc.dram_tensor("wq", [E, EG], BF16, kind="ExternalInput").ap()
    wk = nc.dram_tensor("wk", [E, EG], BF16, kind="ExternalInput").ap()
    wv = nc.dram_tensor("wv", [E, EG], BF16, kind="ExternalInput").ap()
    wo = nc.dram_tensor("wo", [EG, E], BF16, kind="ExternalInput").ap()
    bq = nc.dram_tensor("bq", [128, NJ], F32, kind="ExternalInput").ap()
    negmask = nc.dram_tensor("negmask", [128, NKC], F32, kind="ExternalInput").ap()
    out = nc.dram_tensor("out", [Pq, E], BF16, kind="ExternalOutput").ap()
    if want_kv:
        out_k = nc.dram_tensor("out_k", [EG, Pk], BF16, kind="ExternalOutput").ap()
        out_v = nc.dram_tensor("out_v", [Pk, EG], BF16, kind="ExternalOutput").ap()

    with TileContext(nc) as tc:
        with tc.tile_pool(name="persist", bufs=1) as persist:
            bq_t = persist.tile([128, NJ], F32)
            nc.sync.dma_start(out=bq_t, in_=bq[:, :])
            nm_t = persist.tile([128, NKC], F32)
            nc.sync.dma_start(out=nm_t, in_=negmask[:, :])
            ones_t = persist.tile([1, 128], BF16)
            nc.vector.memset(ones_t, 1.0)
            ones32 = persist.tile([1, 128], F32)
            nc.vector.memset(ones32, 1.0)

            xt = [persist.tile([128, Pk], BF16, name=f"xt{k}") for k in range(KT)]
            wk_sb = [persist.tile([128, EG], BF16, name=f"wk{k}") for k in range(KT)]
            wq_sb = [persist.tile([128, EG], BF16, name=f"wq{k}") for k in range(KT)]
            wv_sb = [persist.tile([128, EG], BF16, name=f"wv{k}") for k in range(KT)]
            wo_sb = [persist.tile([128, E], BF16, name=f"wo{k}") for k in range(NJ)]
            kstage = [persist.tile([128, Pk], BF16, name=f"kst{j}") for j in range(NJ)]
            qT = [persist.tile([128, Pq], BF16, name=f"qT{j}") for j in range(NJ)]
            vhat = [persist.tile([128, HG, D + 1], BF16, name=f"vh{v}") for v in range(NKC)]
            ao = [persist.tile([128, Pq], BF16, name=f"ao{t}") for t in range(PAIRS)]
            wops = [persist.tile([128, E], BF16, name=f"wop{q}")
                    for q in range(QBLK // 128)]

            for k in range(KT):
                nc.sync.dma_start(out=xt[k][:, 0:512],
                                  in_=xT[k * 128:(k + 1) * 128, 0:512])
                nc.sync.dma_start(out=wk_sb[k][:, 0:128],
                                  in_=wk[k * 128:(k + 1) * 128, 0:128])
                nc.sync.dma_start(out=wq_sb[k][:, 0:128],
                                  in_=wq[k * 128:(k + 1) * 128, 0:128])
            for k in range(KT):
                nc.sync.dma_start(out=xt[k][:, 512:Pk],
                                  in_=xT[k * 128:(k + 1) * 128, 512:Pk])
            for k in range(KT):
                nc.sync.dma_start(out=wv_sb[k], in_=wv[k * 128:(k + 1) * 128, :])
            for k in range(KT):
                nc.sync.dma_start(out=wk_sb[k][:, 128:EG],
                                  in_=wk[k * 128:(k + 1) * 128, 128:EG])
                nc.sync.dma_start(out=wq_sb[k][:, 128:EG],
                                  in_=wq[k * 128:(k + 1) * 128, 128:EG])

            with (
                tc.tile_pool(name="p2s", bufs=3) as p2s,
                tc.tile_pool(name="psA", bufs=1, space="PSUM") as psA,
            ):
                def sc_tile():
                    return psA.tile([128, 1024], F32, name="ps_sc", tag="sc", bufs=2)

                def emit_k(j):
                    for off, w in _chunks(Pk):
                        ps = sc_tile()
                        for k in range(KT):
                            nc.tensor.matmul(
                                ps[:, 0:w],
                                wk_sb[k][:, j * 128:(j + 1) * 128],
                                xt[k][:, off:off + w],
                                start=(k == 0), stop=(k == KT - 1),
                            )
                        nc.vector.tensor_copy(kstage[j][:, off:off + w], ps[:, 0:w])
                    if want_kv:
                        nc.sync.dma_start(out=out_k[j * 128:(j + 1) * 128, :],
                                          in_=kstage[j])

                def emit_q(j):
                    for off, w in _chunks(Pq):
                        ps = sc_tile()
                        for k in range(KT):
                            nc.tensor.matmul(
                                ps[:, 0:w],
                                wq_sb[k][:, j * 128:(j + 1) * 128],
                                xt[k][:, off:off + w],
                                start=(k == 0), stop=(k == KT - 1),
                            )
                        nc.vector.tensor_scalar_add(
                            qT[j][:, off:off + w], ps[:, 0:w], bq_t[:, j:j + 1]
                        )

                def emit_v(kc):
                    ps = sc_tile()
                    for k in range(KT):
                        nc.tensor.matmul(
                            ps[:, 0:EG], xt[k][:, kc * 128:(kc + 1) * 128],
                            wv_sb[k][:, 0:EG],
                            start=(k == 0), stop=(k == KT - 1),
                        )
                    nc.vector.tensor_copy(
                        vhat[kc][:, :, 0:D],
                        ps[:, 0:EG].rearrange("p (h d) -> p h d", h=HG),
                    )
                    nc.vector.memset(vhat[kc][:, :, D:D + 1], 1.0)
                    if want_kv:
                        nc.sync.dma_start(
                            out=out_v[kc * 128:(kc + 1) * 128, :],
                            in_=vhat[kc][:, :, 0:D],
                        )

                def emit_scores_exp(t, kc, hh, q0):
                    prows = slice(hh * D, (hh + 1) * D)
                    ps = sc_tile()
                    for off, w in _chunks(QBLK):
                        nc.tensor.matmul(
                            ps[:, off:off + w],
                            kstage[t][prows, kc * 128:(kc + 1) * 128],
                            qT[t][prows, q0 + off:q0 + off + w],
                            start=True, stop=True,
                            tile_position=(hh * D, 0),
                        )
                    ph = p2s.tile([128, QBLK], BF16, name="ph", tag="ph", bufs=6)
                    nc.scalar.activation(
                        ph, ps, mybir.ActivationFunctionType.Exp,
                        bias=nm_t[:, kc:kc + 1], scale=SCALE,
                    )
                    return ph

                def emit_av(t, kc, hh, ph, ps_av):
                    h = 2 * t + hh
                    for off, w in _chunks(QBLK):
                        nc.tensor.matmul(
                            ps_av[hh][:, off:off + w],
                            vhat[kc][:, h, :],
                            ph[:, off:off + w],
                            start=(kc == 0), stop=(kc == NKC - 1),
                        )

                def emit_evac(t, ps_av, q0, last=False):
                    """Evacuate PSUM fast: all four copies first (frees the av
                    accumulator banks for the next pair), then the
                    reciprocals (fast-approx DVE, ~18 significant bits)."""
                    lls = []
                    for hh in range(2):
                        nc.vector.tensor_copy(
                            ao[t][hh * D:(hh + 1) * D, q0:q0 + QBLK],
                            ps_av[hh][0:D, :],
                        )
                        lsb = p2s.tile([1, QBLK], F32, name="lsb", tag="lsb", bufs=2)
                        nc.vector.tensor_copy(lsb, ps_av[hh][D:D + 1, :])
                        lls.append(lsb)
                    rcps = []
                    for hh in range(2):
                        rcp = p2s.tile([1, QBLK], F32, name="rcp", tag="rcp", bufs=2)
                        nc.vector.reciprocal_approx_fast(rcp, lls[hh])
                        rbf = p2s.tile([1, QBLK], BF16, name="rbf", tag="rbf", bufs=2)
                        nc.vector.tensor_copy(rbf, rcp)
                        rcps.append(rbf)
                    return rcps

                def emit_norm_apply(t, rcps, q0):
                    """ps_b = broadcast(1/l) via ones-matmul; ao *= ps_b."""
                    ps_b = sc_tile()
                    for hh in range(2):
                        for off, w in _chunks(QBLK):
                            nc.tensor.matmul(
                                ps_b[hh * D:(hh + 1) * D, off:off + w],
                                ones_t[:, 0:D], rcps[hh][:, off:off + w],
                                start=True, stop=True,
                            )
                    dst = ao[t][:, q0:q0 + QBLK]
                    nc.vector.tensor_mul(dst, dst, ps_b)

                def emit_wo_partial(qt, q0):
                    """wops[qt] = sum over pairs 0..2 of ao[t][:, qt] @ wo[t]."""
                    ps = sc_tile()
                    for fc in range(2):
                        for t in range(PAIRS - 1):
                            nc.tensor.matmul(
                                ps[:, fc * 512:(fc + 1) * 512],
                                ao[t][:, q0 + qt * 128:q0 + (qt + 1) * 128],
                                wo_sb[t][:, fc * 512:(fc + 1) * 512],
                                start=(t == 0), stop=(t == PAIRS - 2),
                            )
                    nc.vector.tensor_copy(wops[qt], ps)

                def emit_wo_final(qt, q0, split=False):
                    """out rows qt = wops[qt] + pair 3's contribution."""
                    t = PAIRS - 1
                    ps = sc_tile()
                    for fc in range(2):
                        nc.tensor.matmul(
                            ps[:, fc * 512:(fc + 1) * 512],
                            ao[t][:, q0 + qt * 128:q0 + (qt + 1) * 128],
                            wo_sb[t][:, fc * 512:(fc + 1) * 512],
                            start=True, stop=True,
                        )
                    osb = p2s.tile([128, E], BF16, name="osb", tag="osb", bufs=3)
                    nc.vector.tensor_add(osb, wops[qt], ps)
                    nc.sync.dma_start(
                        out=out[q0 + qt * 128:q0 + (qt + 1) * 128, :], in_=osb)

                def emit_k_chunk(j, off, w):
                    for coff, cw in _chunks(w):
                        ps = sc_tile()
                        for k in range(KT):
                            nc.tensor.matmul(
                                ps[:, 0:cw],
                                wk_sb[k][:, j * 128:(j + 1) * 128],
                                xt[k][:, off + coff:off + coff + cw],
                                start=(k == 0), stop=(k == KT - 1),
                            )
                        nc.vector.tensor_copy(
                            kstage[j][:, off + coff:off + coff + cw], ps[:, 0:cw])

                # ---- emission schedule ----
                emit_k_chunk(0, 0, min(Pk, 1024))
                emit_q(0)

                # pre[(bi, t, kc)] -> deferred production tasks
                pre = {}
                if Pk > 1024:
                    pre.setdefault((0, 0, 0), []).append(("ktail", 0))
                elif want_kv:
                    nc.sync.dma_start(out=out_k[0:128, :], in_=kstage[0])
                for kc in range(2):
                    pre.setdefault((0, 0, kc), []).append(("v", kc))
                for kc in range(2, NKC):
                    pre.setdefault((0, 0, kc - 2), []).append(("v", kc))
                for t in range(1, PAIRS):
                    pre.setdefault((0, t - 1, 3), []).append(("k", t))
                    pre.setdefault((0, t - 1, 6), []).append(("q", t))

                def run_pre(tasks):
                    for kind, arg in tasks:
                        if kind == "ktail":
                            for off, w in _chunks(Pk, 1024)[1:]:
                                emit_k_chunk(arg, off, w)
                            if want_kv:
                                nc.sync.dma_start(
                                    out=out_k[arg * 128:(arg + 1) * 128, :],
                                    in_=kstage[arg])
                        elif kind == "k":
                            emit_k(arg)
                        elif kind == "q":
                            emit_q(arg)
                        elif kind == "wo":
                            for k in range(NJ):
                                nc.sync.dma_start(
                                    out=wo_sb[k], in_=wo[k * 128:(k + 1) * 128, :])
                        else:
                            emit_v(arg)

                # deferred post-pair tasks (norm_apply, WO partials/finals);
                # consumed two per kc slot during subsequent pairs.
                deferred = []

                def drain(nmax):
                    for _ in range(nmax):
                        if not deferred:
                            return
                        deferred.pop(0)()

                for bi in range(NB):
                    q0 = bi * QBLK
                    if bi == 0:
                        pre.setdefault((0, 0, 5), []).append(("wo", None))
                    for t in range(PAIRS):
                        ps_av = {
                            hh: psA.tile([D + 1, QBLK], F32, name="ps_av",
                                         tag="av", bufs=2)
                            for hh in range(2)
                        }
                        pend = []
                        for kc in range(NKC):
                            run_pre(pre.get((bi, t, kc), ()))
                            if kc >= 3 and not (bi == 0 and t == 0):
                                drain(2)
                            for hh in range(2):
                                ph = emit_scores_exp(t, kc, hh, q0)
                                pend.append((kc, hh, ph))
                                if len(pend) > 2:
                                    kc0, hh0, ph0 = pend.pop(0)
                                    emit_av(t, kc0, hh0, ph0, ps_av)
                        for kc0, hh0, ph0 in pend:
                            emit_av(t, kc0, hh0, ph0, ps_av)
                        last = (bi == NB - 1) and (t == PAIRS - 1)
                        rcps = emit_evac(t, ps_av, q0, last=last)
                        deferred.append(
                            lambda t=t, r=rcps, q=q0: emit_norm_apply(t, r, q))
                        if t == 2:
                            deferred.extend(
                                lambda qt=qt, q=q0: emit_wo_partial(qt, q)
                                for qt in range(QBLK // 128))
                        elif t == PAIRS - 1:
                            tail = bi == NB - 1
                            deferred.extend(
                                lambda qt=qt, q=q0, s=tail: emit_wo_final(qt, q, s)
                                for qt in range(QBLK // 128))
                for f in deferred:
                    f()
    nc.compile()
    return nc


def _make_executor(key):
    """Build the Bass program once and wrap it in a cached jitted shard_map."""
    import jax
    from jax.experimental.shard_map import shard_map
    from jax.sharding import Mesh, PartitionSpec, NamedSharding
    from concourse.bass2jax import (
        _bass_exec_p,
        install_neuronx_cc_hook,
        partition_id_tensor,
    )

    nc = _build_program(*key)
    install_neuronx_cc_hook()
    assert nc.dbg_addr is None
    partition_name = nc.partition_id_tensor.name if nc.partition_id_tensor else None

    in_names, out_names, out_avals, zero_outs = [], [], [], []
    for alloc in nc.m.functions[0].allocations:
        if not isinstance(alloc, mybir.MemoryLocationSet):
            continue
        name = alloc.memorylocations[0].name
        if alloc.kind == "ExternalInput":
            if name != partition_name:
                in_names.append(name)
        elif alloc.kind == "ExternalOutput":
            shape = tuple(alloc.tensor_shape)
            dtype = mybir.dt.np(alloc.dtype)
            out_names.append(name)
            out_avals.append(jax.core.ShapedArray(shape, dtype))
            zero_outs.append(np.zeros(shape, dtype))
    n_params = len(in_names)
    n_outs = len(out_avals)
    all_names = in_names + out_names
    if partition_name is not None:
        all_names = all_names + [partition_name]
    donate = tuple(range(n_params, n_params + n_outs))

    def _body(*args):
        operands = list(args)
        if partition_name is not None:
            operands.append(partition_id_tensor())
        outs = _bass_exec_p.bind(
            *operands,
            out_avals=tuple(out_avals),
            in_names=tuple(all_names),
            out_names=tuple(out_names),
            lowering_input_output_aliases=(),
            sim_require_finite=True,
            sim_require_nnan=True,
            nc=nc,
        )
        return tuple(outs)

    devices = jax.devices()[:N_CORES]
    mesh = Mesh(np.asarray(devices), ("core",))
    in_specs = (PartitionSpec("core"),) * (n_params + n_outs)
    out_specs = (PartitionSpec("core"),) * n_outs
    sharded = jax.jit(
        shard_map(_body, mesh=mesh, in_specs=in_specs, out_specs=out_specs,
                  check_rep=False),
        donate_argnums=donate,
        keep_unused=True,
    )
    sharding = NamedSharding(mesh, PartitionSpec("core"))
    return {
        "jit": sharded, "in_names": in_names, "out_names": out_names,
        "out_avals": out_avals, "zero_outs": zero_outs, "sharding": sharding,
        "jax": jax, "nc": nc,
    }


def get_executor(key):
    ck = ("ex",) + key
    if ck not in _prog_cache:
        _prog_cache[ck] = _make_executor(key)
    return _prog_cache[ck]


def run_spmd(in_maps, key):
    ex = get_executor(key)
    jax = ex["jax"]
    concat_in = [
        np.concatenate([np.asarray(m[name]) for m in in_maps], axis=0)
        for name in ex["in_names"]
    ]
    concat_zeros = [
        np.zeros((N_CORES * z.shape[0], *z.shape[1:]), z.dtype)
        for z in ex["zero_outs"]
    ]
    out_arrs = ex["jit"](*concat_in, *concat_zeros)
    return [
        {
            name: np.asarray(out_arrs[i]).reshape(N_CORES, *ex["out_avals"][i].shape)[c]
            for i, name in enumerate(ex["out_names"])
        }
        for c in range(N_CORES)
    ]


def plan_shapes(mask):
    """Returns (Pk, Pq, want_kv, idxs)."""
    idxs = [np.nonzero(mask[b])[0] for b in range(B)]
    max_m = max((len(ix) for ix in idxs), default=0)
    max_m = max(max_m, 1)
    Pk = ((max_m + 127) // 128) * 128
    nb = max(1, -(-(max_m - OVF) // QBLK))
    Pq = nb * QBLK
    Pk = max(Pk, Pq)   # queries are a prefix of the key rows
    want_kv = max_m > Pq
    return Pk, Pq, want_kv, idxs


def build_in_maps(x, mask, WQ_w, WQ_b, WK_w, WK_b, WV_w, WV_b, WO_w, WO_b):
    x = np.asarray(x, dtype=np.float32)
    mask = np.asarray(mask).astype(bool)
    WQ_w = np.asarray(WQ_w, dtype=np.float32)
    WQ_b = np.asarray(WQ_b, dtype=np.float32)
    WK_w = np.asarray(WK_w, dtype=np.float32)
    WV_w = np.asarray(WV_w, dtype=np.float32)
    WO_w = np.asarray(WO_w, dtype=np.float32)

    Pk, Pq, want_kv, idxs = plan_shapes(mask)
    NKC = Pk // 128

    wq_t = np.ascontiguousarray(WQ_w.T).astype(ml_dtypes.bfloat16)
    wk_t = np.ascontiguousarray(WK_w.T).astype(ml_dtypes.bfloat16)
    wv_t = np.ascontiguousarray(WV_w.T).astype(ml_dtypes.bfloat16)
    wo_t = np.ascontiguousarray(WO_w.T).astype(ml_dtypes.bfloat16)

    in_maps = []
    for c in range(N_CORES):
        b, hg = divmod(c, 2)
        ix = idxs[b]
        m = len(ix)
        xv = np.zeros((E, Pk), dtype=ml_dtypes.bfloat16)
        xv[:, :m] = x[b, ix].T
        cols = slice(hg * EG, (hg + 1) * EG)
        nm = np.full(Pk, -1e6, dtype=np.float32)
        nm[:m] = 0.0
        bq_sl = WQ_b[cols].reshape(NJ, 128).T.astype(np.float32)
        in_maps.append({
            "xT": xv,
            "wq": np.ascontiguousarray(wq_t[:, cols]),
            "wk": np.ascontiguousarray(wk_t[:, cols]),
            "wv": np.ascontiguousarray(wv_t[:, cols]),
            "wo": np.ascontiguousarray(wo_t[cols, :]),
            "bq": np.ascontiguousarray(bq_sl),
            "negmask": np.ascontiguousarray(nm.reshape(NKC, 128).T),
        })
    return in_maps, (Pk, Pq, want_kv), idxs


def combine(results, key, idxs, x, WQ_w, WQ_b, WV_b, WO_w, WO_b):
    """Host-side unshard: sum partials, finish overflow rows, bias+abs+scatter."""
    Pk, Pq, want_kv = key
    bo_eff = (np.asarray(WO_w, np.float32) @ np.asarray(WV_b, np.float32)
              + np.asarray(WO_b, np.float32))
    out = np.zeros((B, S, E), dtype=np.float32)
    for b in range(B):
        ix = idxs[b]
        m = len(ix)
        md = min(m, Pq)
        part = (results[2 * b]["out"][:md].astype(np.float32)
                + results[2 * b + 1]["out"][:md].astype(np.float32)
                + bo_eff[None, :])
        out[b, ix[:md], :] = np.abs(part)
        if m > Pq:
            # overflow rows: full attention in numpy using device K^T / V
            r = np.asarray(x[b], np.float32)[ix[Pq:]]              # [rq, E]
            q = (r @ np.asarray(WQ_w, np.float32).T
                 + np.asarray(WQ_b, np.float32)).reshape(-1, H, D)  # [rq, H, D]
            kT = np.concatenate(
                [results[2 * b]["out_k"].astype(np.float32)[:, :m],
                 results[2 * b + 1]["out_k"].astype(np.float32)[:, :m]],
                axis=0,
            ).reshape(H, D, m)                                       # [H, D, m]
            v = np.concatenate(
                [results[2 * b]["out_v"].astype(np.float32)[:m],
                 results[2 * b + 1]["out_v"].astype(np.float32)[:m]],
                axis=1,
            ).reshape(m, H, D)                                       # [m, H, D]
            sc = np.einsum("qhd,hdm->qhm", q, kT) * SCALE
            p = np.exp(sc)
            o = np.einsum("qhm,mhd->qhd", p, v) / p.sum(-1)[..., None]
            o = o.reshape(-1, H * D) @ np.asarray(WO_w, np.float32).T + bo_eff
            out[b, ix[Pq:], :] = np.abs(o)
    return out


def kernel(x, mask, WQ_w, WQ_b, WK_w, WK_b, WV_w, WV_b, WO_w, WO_b):
    mask = np.asarray(mask).astype(bool)
    in_maps, key, idxs = build_in_maps(
        x, mask, WQ_w, WQ_b, WK_w, WK_b, WV_w, WV_b, WO_w, WO_b)
    results = run_spmd(in_maps, key)
    return combine(results, key, idxs, x, WQ_w, WQ_b, WV_b, WO_w, WO_b)
